# revision 92
# baseline (speedup 1.0000x reference)
"""Multi-head attention (B=1, S=4096, D=768, 12 heads) on 8 trn2 cores.

Sharding: tensor-parallel by heads, balanced with sequence splits.
Core c owns: head A = c (all 4096 query rows) and head B = 8 + c//2
(query-row half c%2).  Each core computes q/k/v for its two heads, full
S x S attention for its share, and its heads' partial contribution to
the output projection (row-parallel split of w_proj).  The host sums
the per-core projection partials and adds the bias.

Device layout: d-on-partitions ("transposed") everywhere.  Scores are
computed as S^T[t, s] = K^T.T @ Q^T per 128-key chunk; exp split across
ScalarE + the custom DVE op (logits are bounded, no max subtraction);
the AV matmul uses a [V | ones] stationary operand so rows 0-63 of PSUM
accumulate the UNNORMALIZED O^T and rows 64-127 the softmax denominator
in the same pass.  Normalization (a per-query scalar that commutes with
the projection) happens on the HOST: the kernel ships unnormalized
projection partials plus the denominator rows.

The attention loop runs "sweeps" that each process TWO units at once -
one on PE row-groups 0-1 (SBUF partitions 0-63) and one on row-groups
2-3 (partitions 64-127) - so the K=64 score matmuls run pairwise
concurrent on the PE array.  Head A's second half is paired with itself
via a partition-shifted duplicate of Q^T/K^T.  The inner loop is
batched in pairs of key-chunks so the AV accumulations issue as
same-PSUM-bank runs (avoids the per-matmul bank-switch micro-idle), and
each sweep's drain is emitted inside the next sweep so it never blocks
the exp pipeline.  Head B's query rows are always LOCAL x chunks 0-3:
odd cores get their chunks rolled by 4 host-side (key order is
irrelevant to softmax; the host un-permutes the output blocks), which
removes any duplicate query input.  All DMA is chunk-major fully
contiguous (6KB/partition elements), spread across the gpsimd + scalar
hardware queues just-in-time against the sweep-0 weave, with the slow
sync queue reserved for V transposes and output halves.  All matmuls
are bf16 with fp32 PSUM accumulation.
"""

import numpy as np
import ml_dtypes

import concourse.bass as bass
import concourse.mybir as mybir
import concourse.tile as tile
from concourse import bacc
from concourse.bass_utils import run_bass_kernel_spmd

BF16 = mybir.dt.bfloat16
F32 = mybir.dt.float32
ts = bass.ts
ds = bass.ds

S = 4096
D = 768
NH = 12
HD = 64
NCORES = 8
SU = 2048          # rows per unit
PO = D // 128      # 6 e-chunks
NT = S // 128      # 32 key chunks
NC8 = S // 512     # 8 column chunks
SCALE = HD ** -0.5

_CACHE: dict = {}

# --- custom DVE exp op: out_uint16 = bf16 bits of 2^((x - 64)/128) ---------
# Magic-constant round to the 128-grid + quadratic mantissa correction,
# emitted through the fp32->uint16 value cast.  The -64 window shift (a
# global 2^-0.5 factor on all exp values) cancels in the softmax
# normalization; the ScalarE branch matches it via the activation bias.
EXP_M = 1.5 * 2**30
EXP_Q0 = 16180.991964579287
EXP_Q1 = 0.9950478871994926
EXP_Q2 = 0.0026875086476569427
EXP_SCALE = float(np.log(2) / 128.0)
EXP_BIAS = float(-np.log(2) / 2.0)
LOG2E_128 = float(128.0 / np.log(2))


def _expb_ref(in0, in1, s0, s1, imm2):
    f32 = np.float32
    a = (in0.astype(f32) + f32(s0)).astype(f32)
    u = (a - f32(s0)).astype(f32)
    z = (in0.astype(f32) - u).astype(f32)
    m2 = (((z * f32(s1)).astype(f32) + f32(imm2)).astype(f32) * z).astype(f32)
    return ((u + m2).astype(f32) + in1.astype(f32)).astype(f32)


def _expb_op():
    from concourse import dve_ops
    from concourse.dve_spec import Spec, Src0, C0, C1, C2, C3, lower, _spill_c3_to_src1
    from concourse.dve_uop import DveOpSpec

    for op in dve_ops.OPS:
        if op.name == "EXPB_ANT":
            return op
    a = Src0 + C0
    u = a - C0
    z = Src0 - u
    m2 = (z * C1 + C2) * z
    body = _spill_c3_to_src1((u + m2) + C3)
    spec = Spec(body=body, reference=_expb_ref)
    row = dve_ops._CUSTOM_DVE_ROW_BASE + len(dve_ops.OPS)
    dve_ops._SUB_OPCODE_FOR_NAME["EXPB_ANT"] = row
    shas = {}
    for ver in ("v3", "v4"):
        try:
            uops = lower(spec, ver=ver)
            shas[ver] = DveOpSpec(
                name="EXPB_ANT", opcode=row, uops=uops, rd1_en=True
            ).sha(ver)
        except Exception:
            pass
    op = dve_ops.DveOp("EXPB_ANT", spec, subdim=False, uops_sha=shas)
    dve_ops.OPS.append(op)
    dve_ops.CUSTOM_DVE_SPECS["EXPB_ANT"] = spec
    return op


def _emit(nc):
    # chunk-major inputs, flattened so every DMA element is one fully
    # contiguous 6KB-per-partition block (1KB elements run ~5x slower)
    xT = nc.dram_tensor("xT", [128, NC8, PO * 512], BF16, kind="ExternalInput")
    wq = nc.dram_tensor("wq", [128, PO * 128], BF16, kind="ExternalInput")
    wk = nc.dram_tensor("wk", [128, PO * 128], BF16, kind="ExternalInput")
    wv = nc.dram_tensor("wv", [128, PO * 128], BF16, kind="ExternalInput")
    wp = nc.dram_tensor("wp", [64, 2 * D], BF16, kind="ExternalInput")
    yTa = nc.dram_tensor("yTa", [128, NC8, PO * 512], BF16, kind="ExternalOutput")
    yTb = nc.dram_tensor("yTb", [128, 4, PO * 512], BF16, kind="ExternalOutput")
    # per-query softmax denominators (12 column blocks); host divides
    den = nc.dram_tensor("den", [1, 12 * 512], F32, kind="ExternalOutput")

    with tile.TileContext(nc) as tc:
        with (
            tc.tile_pool(name="persist", bufs=1) as pp,
            tc.tile_pool(name="work", bufs=4) as wkp,
            tc.tile_pool(name="ps", bufs=2, space="PSUM") as psp,
        ):
            # ---------------- persistent SBUF tensors ----------------
            xT_sb = pp.tile([128, NC8, PO * 512], BF16)
            wq_sb = pp.tile([128, PO * 128], BF16)
            wk_sb = pp.tile([128, PO * 128], BF16)
            wv_sb = pp.tile([128, PO * 128], BF16)
            wp_sb = pp.tile([64, 2 * D], BF16)
            QT_sb = pp.tile([128, S], BF16)      # 0:64 A (full S); 64:128 B (cols 0:SU) + A-dup (cols 3072:4096)
            KT_sb = pp.tile([128, S], BF16)      # 0:64 A, 64:128 B
            KT2_sb = pp.tile([128, S], BF16)     # 64:128 = copy of A rows (for self-pairing)
            VT_sb = pp.tile([128, S], BF16)      # V^T staging for the DMA transpose
            V_sb = pp.tile([128, NT, 256], BF16)  # [V_A |ones| V_B |ones]
            ot_u0 = pp.tile([64, SU], BF16)
            ot_u1 = pp.tile([64, SU], BF16)
            ot_u2 = pp.tile([64, SU], BF16)
            ot_tiles = {"u0": ot_u0, "u1": ot_u1, "u2": ot_u2}
            den_sb = pp.tile([128, 12 * 512], F32)  # rows 64:128 used

            # DMA split: gpsimd + scalar queues are the fast (hardware-
            # dynamic) paths — they carry ALL bulk input, interleaved so the
            # just-in-time order (wq, x0, wk, x1, wv, x2, ...) lands first.
            # The slow sync queue carries only transposes/rlo/output halves.
            # First two chunks split in halves across both queues so the
            # sweep-0 pipeline can start sooner.
            HX = PO * 256
            nc.gpsimd.dma_start(wq_sb[:], wq.ap())
            nc.scalar.dma_start(xT_sb[:, 0, HX : 2 * HX], xT.ap()[:, 0, HX : 2 * HX])
            nc.gpsimd.dma_start(xT_sb[:, 0, 0:HX], xT.ap()[:, 0, 0:HX])
            nc.scalar.dma_start(wk_sb[:], wk.ap())
            nc.gpsimd.dma_start(xT_sb[:, 1, 0:HX], xT.ap()[:, 1, 0:HX])
            nc.scalar.dma_start(xT_sb[:, 1, HX : 2 * HX], xT.ap()[:, 1, HX : 2 * HX])
            nc.gpsimd.dma_start(wv_sb[:], wv.ap())
            # remaining chunks: x2 split across both fast queues, rest
            # alternating (sync must stay clear for the V transposes)
            nc.scalar.dma_start(xT_sb[:, 2, HX : 2 * HX], xT.ap()[:, 2, HX : 2 * HX])
            nc.gpsimd.dma_start(xT_sb[:, 2, 0:HX], xT.ap()[:, 2, 0:HX])
            nc.scalar.dma_start(xT_sb[:, 3], xT.ap()[:, 3])
            nc.gpsimd.dma_start(xT_sb[:, 4], xT.ap()[:, 4])
            nc.scalar.dma_start(xT_sb[:, 5], xT.ap()[:, 5])
            nc.gpsimd.dma_start(xT_sb[:, 6], xT.ap()[:, 6])
            nc.scalar.dma_start(xT_sb[:, 7], xT.ap()[:, 7])
            nc.gpsimd.dma_start(wp_sb[:], wp.ap())

            q0_sb = pp.tile([128, 1], F32)
            bias_sb = pp.tile([128, 1], F32)
            nc.vector.memset(V_sb[:, :, 64:128], 1.0)
            nc.vector.memset(V_sb[:, :, 192:256], 1.0)
            # HAM pre-warm: dense matmuls while the input DMAs stream so the
            # PE clock is at 2.4 GHz when real work starts
            warm_sb = pp.tile([128, 128], BF16)
            nc.vector.memset(warm_sb[:], 0.0)
            warm_ps = psp.tile([128, 512], F32, tag="aux", name="warm_ps")
            for i in range(44):
                nc.tensor.matmul(
                    warm_ps[:, 0:128], lhsT=warm_sb[:], rhs=warm_sb[:],
                    start=(i == 0), stop=(i == 43),
                )
            nc.vector.memset(q0_sb[:], EXP_Q0)
            nc.vector.memset(bias_sb[:], EXP_BIAS)
            expb = _expb_op()

            # ---------------- q/k/v projection blocks -----------------
            pfx_alt = [0]

            def pfx_copy(dst, srcv):
                pfx_alt[0] ^= 1
                if pfx_alt[0]:
                    nc.vector.tensor_copy(dst, srcv)
                else:
                    nc.scalar.copy(dst, srcv)

            def emit_qt_pair(n):
                # n<4: heads A+B queries for LOCAL block n in one full-width
                # chain (bh=1 cores get their x chunks rolled by 4 host-side:
                # softmax over keys is order-invariant and the host
                # un-permutes the output blocks).  n>=4: head A only, blocks n
                # and n+1 as two column-tiled concurrent chains sharing the
                # bank (each chain has its own start/stop; the pending-zero
                # clear is per partition range).
                q_ps = psp.tile([128, 512], F32, tag="aux", name="q_ps")
                for po in range(PO):
                    if n < 4:
                        nc.tensor.matmul(
                            q_ps[:],
                            lhsT=wq_sb[:, ts(po, 128)],
                            rhs=xT_sb[:, n, ts(po, 512)],
                            start=(po == 0),
                            stop=(po == PO - 1),
                        )
                    else:
                        nc.tensor.matmul(
                            q_ps[0:64, :],
                            lhsT=wq_sb[:, ds(po * 128, 64)],
                            rhs=xT_sb[:, n, ts(po, 512)],
                            start=(po == 0),
                            stop=(po == PO - 1),
                            skip_group_check=True,
                        )
                        nc.tensor.matmul(
                            q_ps[64:128, :],
                            lhsT=wq_sb[:, ds(po * 128, 64)],
                            rhs=xT_sb[:, n + 1, ts(po, 512)],
                            start=(po == 0),
                            stop=(po == PO - 1),
                            skip_group_check=True,
                        )
                if n < 4:
                    pfx_copy(QT_sb[:, ts(n, 512)], q_ps[:])
                else:
                    pfx_copy(QT_sb[0:64, ts(n, 512)], q_ps[0:64, :])
                    pfx_copy(QT_sb[0:64, ts(n + 1, 512)], q_ps[64:128, :])

            def emit_kt_block(n):
                k_ps = psp.tile([128, 512], F32, tag="aux", name="k_ps")
                for po in range(PO):
                    nc.tensor.matmul(
                        k_ps[:, 0:512],
                        lhsT=wk_sb[:, ts(po, 128)],
                        rhs=xT_sb[:, n, ts(po, 512)],
                        start=(po == 0),
                        stop=(po == PO - 1),
                    )
                pfx_copy(KT_sb[:, ts(n, 512)], k_ps[:, 0:512])

            def emit_vt_block(n):
                v_ps = psp.tile([128, 512], F32, tag="aux", name="v_ps")
                for po in range(PO):
                    nc.tensor.matmul(
                        v_ps[:, 0:512],
                        lhsT=wv_sb[:, ts(po, 128)],
                        rhs=xT_sb[:, n, ts(po, 512)],
                        start=(po == 0),
                        stop=(po == PO - 1),
                    )
                pfx_copy(VT_sb[:, ts(n, 512)], v_ps[:, 0:512])
                vv = V_sb[:, 4 * n : 4 * n + 4, :]
                nc.sync.dma_start_transpose(vv[:, :, 0:64], VT_sb[0:64, ts(n, 512)])
                nc.sync.dma_start_transpose(vv[:, :, 128:192], VT_sb[64:128, ts(n, 512)])

            # ---------------- attention sweeps ----------------
            # unit specs: (vbase, wp_idx, ydst, ycolbase)
            USPEC = {
                "u0": (0, 0, yTa, 0),
                "u1": (0, 0, yTa, SU),
                "u2": (128, 1, yTb, 0),
            }
            # sweeps: (L, R) sides: (unit, kt_tile, slot, qt_abs_col, ot_local_col)
            sweeps = []
            for sb in range(4):
                sweeps.append(
                    (
                        ("u0", KT_sb, 0, sb * 512, sb * 512),
                        ("u2", KT_sb, 64, sb * 512, sb * 512),
                    )
                )
            for sb in range(2):
                sweeps.append(
                    (
                        ("u1", KT_sb, 0, SU + sb * 512, sb * 512),
                        ("u1", KT2_sb, 64, SU + 1024 + sb * 512, 1024 + sb * 512),
                    )
                )

            proj_q = []
            drain_q = []
            cp_alt = [0]
            oq_alt = [0]
            pj_alt = [0]
            dq_alt = [0]
            rhi_last = [None]

            tw_holder = []
            yst_holder = [None]

            def emit_proj_chunk(u, oe, col, tail=False):
                _vb, wpi, ydst, ybase = USPEC[u]
                # in the tail the score banks are idle: rotate pj through four
                # banks (aux + scL) so the MM->copy loop pipelines deeper
                if tail:
                    pj_alt[0] ^= 1
                    tag = "aux" if pj_alt[0] else "scL"
                else:
                    tag = "aux"
                pj = psp.tile([128, 512], F32, tag=tag, name="pj")
                nc.tensor.matmul(
                    pj[:],
                    lhsT=wp_sb[:, ds(wpi * D + oe * 128, 128)],
                    rhs=ot_tiles[u][0:64, ds(col, 512)],
                    start=True,
                    stop=True,
                )
                if oe == 0:
                    yst_holder[0] = wkp.tile(
                        [128, PO * 512], BF16, tag="yst", name="yst", bufs=2
                    )
                yst = yst_holder[0]
                # balance PSUM->SBUF copies evenly across the two exp engines
                # (both carry 32 exps + 2 drain copies per sweep)
                if tail:
                    eng = nc.scalar if oe % 2 == 0 else nc.vector
                else:
                    oq_alt[0] ^= 1
                    eng = nc.vector if oq_alt[0] == 0 else nc.scalar
                if eng is nc.scalar:
                    nc.scalar.copy(yst[:, ds(oe * 512, 512)], pj[:])
                else:
                    nc.vector.tensor_copy(yst[:, ds(oe * 512, 512)], pj[:])
                if oe == PO - 1:
                    # contiguous output DMA per (unit, column-block) group,
                    # split in halves across both queues for 2x drain rate
                    jc = (col + ybase) // 512
                    half = PO * 512 // 2
                    nc.sync.dma_start(
                        ydst.ap()[:, jc, 0:half], yst[:, 0:half]
                    )
                    nc.gpsimd.dma_start(
                        ydst.ap()[:, jc, half : 2 * half], yst[:, half : 2 * half]
                    )

            def pump(k):
                for _ in range(k):
                    if proj_q:
                        proj_q.pop(0)()

            # pre-sweep: block-0 projections (just-in-time minimum)
            emit_qt_pair(0)
            emit_kt_block(0)
            emit_vt_block(0)

            for si, (Lside, Rside) in enumerate(sweeps):
                if si == 1:
                    # duplicate for head-A self-pairing (after KT complete)
                    nc.gpsimd.dma_start(KT2_sb[64:128, :], KT_sb[0:64, :])
                elif si == 2:
                    # Q duplicate (after the sweep-1-woven qt(6|7) completes)
                    nc.gpsimd.dma_start(
                        QT_sb[64:128, SU + 1024 : S], QT_sb[0:64, SU + 1024 : S]
                    )
                avL = psp.tile([128, 512], F32, tag="av", name="avL")
                avR = psp.tile([128, 512], F32, tag="av", name="avR")

                def emit_av_pair(p0, p1):
                    # same-bank runs: avL(t0), avL(t1), avR(t0), avR(t1) —
                    # consecutive accumulations into one PSUM bank issue
                    # back-to-back with no bank-switch micro-idle
                    for idx, (u, av) in enumerate(((Lside[0], avL), (Rside[0], avR))):
                        vbase = USPEC[u][0]
                        for t, ptl, ptr in (p0, p1):
                            pt = ptl if idx == 0 else ptr
                            nc.tensor.matmul(
                                av,
                                lhsT=V_sb[:, t, vbase : vbase + 128],
                                rhs=pt[:, 0:512],
                                start=(t == 0),
                                stop=(t == NT - 1),
                            )

                pend = []
                for t in range(NT):
                    if t == 1 and drain_q:
                        # prior sweep's drain emits here so its recip/mul ops
                        # queue BEHIND this sweep's first exps on DVE (the sc
                        # bank recycle path stays low-latency at sweep start)
                        drain_q.pop(0)()
                    # qkv generation woven just-in-time into sweep 0: K block
                    # m+1 lands 2 chunks before its scores need it, so a DMA
                    # wait parks the PE queue as late as possible
                    if si == 0:
                        m = t // 4
                        if t % 4 == 1 and m < 7:
                            emit_kt_block(m + 1)
                        elif t % 4 == 2 and m < 7:
                            emit_vt_block(m + 1)
                        elif t == 4:
                            emit_qt_pair(1)
                    elif si == 1:
                        if t == 0:
                            emit_qt_pair(2)
                        elif t == 4:
                            emit_qt_pair(3)
                        elif t == 8:
                            emit_qt_pair(4)
                        elif t == 12:
                            emit_qt_pair(6)
                    # sweeps >=2 have no qkv weaves, so the aux banks are only
                    # lightly used (12 proj chunks): borrow them for odd-t
                    # L-scores, deepening the scL rotation to ~4 chunks.  The
                    # score then waits exp(t-4) instead of exp(t-2), which
                    # absorbs the exp-queue jitter that was stalling the PE.
                    if si >= 2 and t % 2 == 1:
                        scL = psp.tile([128, 512], F32, tag="aux", name="scLx")
                    else:
                        scL = psp.tile([128, 512], F32, tag="scL", name="scL")
                    scR = psp.tile([128, 512], F32, tag="scR", name="scR")
                    for (u, ktt, slot, qcol, _ocol), sct in ((Lside, scL), (Rside, scR)):
                        nc.tensor.matmul(
                            sct[:, 0:512],
                            lhsT=ktt[slot : slot + 64, ts(t, 128)],
                            rhs=QT_sb[slot : slot + 64, ds(qcol, 512)],
                            start=True,
                            stop=True,
                        )
                    ptL = wkp.tile([128, 512], BF16, tag="ptL", name="ptL", bufs=8)
                    ptR = wkp.tile([128, 512], BF16, tag="ptR", name="ptR", bufs=8)
                    # no bias: the 2^0.5 factor vs the DVE branch's window is a
                    # per-unit global scale that cancels in the normalization
                    nc.scalar.activation(
                        ptL[:, 0:512],
                        scL[:, 0:512],
                        mybir.ActivationFunctionType.Exp,
                        scale=EXP_SCALE,
                    )
                    nc.vector._custom_dve(
                        expb,
                        out=ptR[:, 0:512].bitcast(mybir.dt.uint16),
                        in0=scR[:, 0:512],
                        in1=q0_sb[:],
                        s0=EXP_M,
                        s1=EXP_Q2,
                        imm2=EXP_Q1,
                    )
                    pend.append((t, ptL, ptR))
                    if t % 2 == 1:
                        # AVs lag two chunk-pairs behind the scores so their
                        # exp inputs are done at issue: every PE instruction is
                        # dep-free (throughput-bound, not exp-latency-bound)
                        if len(pend) >= 6:
                            emit_av_pair(pend.pop(0), pend.pop(0))
                        pump(1)
                while len(pend) >= 2:
                    emit_av_pair(pend.pop(0), pend.pop(0))

                # drain: stage UNNORMALIZED O^T + the denominator row; the
                # host divides (softmax normalization is a per-query scalar
                # that commutes with the projection).  Deferred into the NEXT
                # sweep (t==1) so these copies don't block its exps.
                def emit_drain(sides=(Lside, Rside), avs=(avL, avR)):
                    for (u, _ktt, _slot, _qcol, ocol), av in zip(sides, avs):
                        _vb, _wpi, _yd, ybase = USPEC[u]
                        jc = (ocol + ybase) // 512 + (8 if u == "u2" else 0)
                        cp_alt[0] ^= 1
                        e1 = nc.scalar if cp_alt[0] else nc.vector
                        e2 = nc.vector if cp_alt[0] else nc.scalar
                        if e1 is nc.scalar:
                            nc.scalar.copy(
                                ot_tiles[u][0:64, ds(ocol, 512)], av[0:64, :]
                            )
                        else:
                            nc.vector.tensor_copy(
                                ot_tiles[u][0:64, ds(ocol, 512)], av[0:64, :]
                            )
                        if e2 is nc.scalar:
                            nc.scalar.copy(
                                den_sb[64:128, ds(jc * 512, 512)], av[64:128, :]
                            )
                        else:
                            nc.vector.tensor_copy(
                                den_sb[64:128, ds(jc * 512, 512)], av[64:128, :]
                            )
                        for oe in range(PO):
                            proj_q.append(
                                lambda tail=False, u=u, oe=oe, col=ocol: (
                                    emit_proj_chunk(u, oe, col, tail)
                                )
                            )

                drain_q.append(emit_drain)
            # last sweep's deferred drain
            while drain_q:
                drain_q.pop(0)()
            # ship the denominator row (one partition's worth)
            nc.sync.dma_start(den.ap(), den_sb[64:65, :])
            # tail: bridge the last sweep's drain latency with filler matmuls
            # that DEPEND on the drain's output (otherwise the scheduler
            # hoists them ahead of the drain and they fill nothing)
            tw_holder.append(psp.tile([128, 512], F32, tag="scR", name="tw"))
            for _ in range(10):
                nc.tensor.matmul(
                    tw_holder[0][:, 0:448],
                    lhsT=ot_u1[0:64, 0:128],
                    rhs=ot_u1[0:64, 1536:1984],
                    start=True, stop=True,
                )
            while proj_q:
                proj_q.pop(0)(tail=True)

    nc.compile()
    return nc


def _build():
    if "nc" not in _CACHE:
        nc = bacc.Bacc(None, target_bir_lowering=False, debug=False)
        _CACHE["nc"] = _emit(nc)
    return _CACHE["nc"]


def _prep_inputs(x, w_qkv, w_proj):
    bf = ml_dtypes.bfloat16
    xs = np.ascontiguousarray(x.reshape(S, D).T).astype(bf)  # [D, S]
    # chunk-major [pi, n, po*512+s]: element = xT[po*128+pi, n*512+s]
    x8 = np.ascontiguousarray(
        xs.reshape(PO, 128, NC8, 512).transpose(1, 2, 0, 3).reshape(128, NC8, PO * 512)
    )
    in_maps = []
    for c in range(NCORES):
        ha = c
        hb = 8 + c // 2
        bh = c % 2
        rows_q = lambda h: w_qkv[h * HD : (h + 1) * HD, :]
        rows_k = lambda h: w_qkv[D + h * HD : D + (h + 1) * HD, :]
        rows_v = lambda h: w_qkv[2 * D + h * HD : 2 * D + (h + 1) * HD, :]
        qs = SCALE * LOG2E_128
        wq_c = np.concatenate([rows_q(ha) * qs, rows_q(hb) * qs], 0).T  # [D, 128]
        wk_c = np.concatenate([rows_k(ha), rows_k(hb)], 0).T
        wv_c = np.concatenate([rows_v(ha), rows_v(hb)], 0).T
        wp_c = np.stack(
            [w_proj[:, ha * HD : (ha + 1) * HD].T, w_proj[:, hb * HD : (hb + 1) * HD].T],
            axis=1,
        )  # [64, 2, D]
        shuf = lambda w: np.ascontiguousarray(
            w.reshape(PO, 128, 128).transpose(1, 0, 2).reshape(128, PO * 128)
        ).astype(bf)
        # local chunk j = global chunk (j + 4*bh) % 8, so head-B's query rows
        # are always local chunks 0-3 (key order is irrelevant to softmax)
        x8c = np.roll(x8, -4 * bh, axis=1) if bh else x8
        in_maps.append(
            {
                "xT": np.ascontiguousarray(x8c),
                "wq": shuf(wq_c),
                "wk": shuf(wk_c),
                "wv": shuf(wv_c),
                "wp": np.ascontiguousarray(wp_c.reshape(64, 2 * D)).astype(bf),
            }
        )
    return in_maps


def _combine(results, b_proj):
    yT = np.zeros((D, S), np.float32)
    for c in range(NCORES):
        bh = c % 2
        dn = results[c]["den"].astype(np.float32).reshape(12, 512)
        ya = results[c]["yTa"].astype(np.float32)  # [128, 8, 6*512] local order
        ya = ya.reshape(128, NC8, PO, 512) / dn[None, 0:8, None, :]
        if bh:
            ya = np.roll(ya, 4 * bh, axis=1)  # local block j -> global (j+4)%8
        yT += ya.transpose(2, 0, 1, 3).reshape(D, S)
        yb = results[c]["yTb"].astype(np.float32)  # [128, 4, 6*512]
        yb = yb.reshape(128, 4, PO, 512) / dn[None, 8:12, None, :]
        yT[:, bh * SU : (bh + 1) * SU] += yb.transpose(2, 0, 1, 3).reshape(D, SU)
    y = yT.T + b_proj.astype(np.float32)[None, :]
    return y.reshape(1, 64, 64, D).astype(np.float32)


def kernel(x, w_qkv, w_proj, b_proj, _trace=False, _trace_kwargs=None):
    x = np.asarray(x, np.float32)
    w_qkv = np.asarray(w_qkv, np.float32)
    w_proj = np.asarray(w_proj, np.float32)
    b_proj = np.asarray(b_proj, np.float32)

    nc = _build()
    in_maps = _prep_inputs(x, w_qkv, w_proj)
    res = run_bass_kernel_spmd(
        nc, in_maps, core_ids=list(range(NCORES)), trace=_trace,
        **(_trace_kwargs or {}),
    )
    out = _combine(res.results, b_proj)
    if _trace:
        return out, res
    return out


# revision 93
# speedup vs baseline: 1.0156x; 1.0156x over previous
"""Multi-head attention (B=1, S=4096, D=768, 12 heads) on 8 trn2 cores.

Sharding: tensor-parallel by heads, balanced with sequence splits.
Core c owns: head A = c (all 4096 query rows) and head B = 8 + c//2
(query-row half c%2).  Each core computes q/k/v for its two heads, full
S x S attention for its share, and its heads' partial contribution to
the output projection (row-parallel split of w_proj).  The host sums
the per-core projection partials and adds the bias.

Device layout: d-on-partitions ("transposed") everywhere.  Scores are
computed as S^T[t, s] = K^T.T @ Q^T per 128-key chunk; exp split across
ScalarE + the custom DVE op (logits are bounded, no max subtraction);
the AV matmul uses a [V | ones] stationary operand so rows 0-63 of PSUM
accumulate the UNNORMALIZED O^T and rows 64-127 the softmax denominator
in the same pass.  Normalization (a per-query scalar that commutes with
the projection) happens on the HOST: the kernel ships unnormalized
projection partials plus the denominator rows.

The attention loop runs "sweeps" that each process TWO units at once -
one on PE row-groups 0-1 (SBUF partitions 0-63) and one on row-groups
2-3 (partitions 64-127) - so the K=64 score matmuls run pairwise
concurrent on the PE array.  Head A's second half is paired with itself
via a partition-shifted duplicate of Q^T/K^T.  The inner loop is
batched in pairs of key-chunks so the AV accumulations issue as
same-PSUM-bank runs (avoids the per-matmul bank-switch micro-idle), and
each sweep's drain is emitted inside the next sweep so it never blocks
the exp pipeline.  Head B's query rows are always LOCAL x chunks 0-3:
odd cores get their chunks rolled by 4 host-side (key order is
irrelevant to softmax; the host un-permutes the output blocks), which
removes any duplicate query input.  All DMA is chunk-major fully
contiguous (6KB/partition elements), spread across the gpsimd + scalar
hardware queues just-in-time against the sweep-0 weave, with the slow
sync queue reserved for V transposes and output halves.  All matmuls
are bf16 with fp32 PSUM accumulation.
"""

import numpy as np
import ml_dtypes

import concourse.bass as bass
import concourse.mybir as mybir
import concourse.tile as tile
from concourse import bacc
from concourse.bass_utils import run_bass_kernel_spmd

BF16 = mybir.dt.bfloat16
F32 = mybir.dt.float32
ts = bass.ts
ds = bass.ds

S = 4096
D = 768
NH = 12
HD = 64
NCORES = 8
SU = 2048          # rows per unit
PO = D // 128      # 6 e-chunks
NT = S // 128      # 32 key chunks
NC8 = S // 512     # 8 column chunks
SCALE = HD ** -0.5

_CACHE: dict = {}

# --- custom DVE exp op: out_uint16 = bf16 bits of 2^((x - 64)/128) ---------
# Magic-constant round to the 128-grid + quadratic mantissa correction,
# emitted through the fp32->uint16 value cast.  The -64 window shift (a
# global 2^-0.5 factor on all exp values) cancels in the softmax
# normalization; the ScalarE branch matches it via the activation bias.
EXP_M = 1.5 * 2**30
EXP_Q0 = 16180.991964579287
EXP_Q1 = 0.9950478871994926
EXP_Q2 = 0.0026875086476569427
EXP_SCALE = float(np.log(2) / 128.0)
EXP_BIAS = float(-np.log(2) / 2.0)
LOG2E_128 = float(128.0 / np.log(2))


def _expb_ref(in0, in1, s0, s1, imm2):
    f32 = np.float32
    a = (in0.astype(f32) + f32(s0)).astype(f32)
    u = (a - f32(s0)).astype(f32)
    z = (in0.astype(f32) - u).astype(f32)
    m2 = (((z * f32(s1)).astype(f32) + f32(imm2)).astype(f32) * z).astype(f32)
    return ((u + m2).astype(f32) + in1.astype(f32)).astype(f32)


def _expb_op():
    from concourse import dve_ops
    from concourse.dve_spec import Spec, Src0, C0, C1, C2, C3, lower, _spill_c3_to_src1
    from concourse.dve_uop import DveOpSpec

    for op in dve_ops.OPS:
        if op.name == "EXPB_ANT":
            return op
    a = Src0 + C0
    u = a - C0
    z = Src0 - u
    m2 = (z * C1 + C2) * z
    body = _spill_c3_to_src1((u + m2) + C3)
    spec = Spec(body=body, reference=_expb_ref)
    row = dve_ops._CUSTOM_DVE_ROW_BASE + len(dve_ops.OPS)
    dve_ops._SUB_OPCODE_FOR_NAME["EXPB_ANT"] = row
    shas = {}
    for ver in ("v3", "v4"):
        try:
            uops = lower(spec, ver=ver)
            shas[ver] = DveOpSpec(
                name="EXPB_ANT", opcode=row, uops=uops, rd1_en=True
            ).sha(ver)
        except Exception:
            pass
    op = dve_ops.DveOp("EXPB_ANT", spec, subdim=False, uops_sha=shas)
    dve_ops.OPS.append(op)
    dve_ops.CUSTOM_DVE_SPECS["EXPB_ANT"] = spec
    return op


def _emit(nc):
    # chunk-major inputs, flattened so every DMA element is one fully
    # contiguous 6KB-per-partition block (1KB elements run ~5x slower)
    xT = nc.dram_tensor("xT", [128, NC8, PO * 512], BF16, kind="ExternalInput")
    wq = nc.dram_tensor("wq", [128, PO * 128], BF16, kind="ExternalInput")
    wk = nc.dram_tensor("wk", [128, PO * 128], BF16, kind="ExternalInput")
    wv = nc.dram_tensor("wv", [128, PO * 128], BF16, kind="ExternalInput")
    wp = nc.dram_tensor("wp", [64, 2 * D], BF16, kind="ExternalInput")
    yTa = nc.dram_tensor("yTa", [128, NC8, PO * 512], BF16, kind="ExternalOutput")
    yTb = nc.dram_tensor("yTb", [128, 4, PO * 512], BF16, kind="ExternalOutput")
    # per-query softmax denominators (12 column blocks); host divides
    den = nc.dram_tensor("den", [1, 12 * 512], F32, kind="ExternalOutput")

    with tile.TileContext(nc) as tc:
        with (
            tc.tile_pool(name="persist", bufs=1) as pp,
            tc.tile_pool(name="work", bufs=4) as wkp,
            tc.tile_pool(name="ps", bufs=2, space="PSUM") as psp,
        ):
            # ---------------- persistent SBUF tensors ----------------
            xT_sb = pp.tile([128, NC8, PO * 512], BF16)
            wq_sb = pp.tile([128, PO * 128], BF16)
            wk_sb = pp.tile([128, PO * 128], BF16)
            wv_sb = pp.tile([128, PO * 128], BF16)
            wp_sb = pp.tile([64, 2 * D], BF16)
            QT_sb = pp.tile([128, S], BF16)      # 0:64 A (full S); 64:128 B (cols 0:SU) + A-dup (cols 3072:4096)
            KT_sb = pp.tile([128, S], BF16)      # 0:64 A, 64:128 B
            KT2_sb = pp.tile([128, S], BF16)     # 64:128 = copy of A rows (for self-pairing)
            VT_sb = pp.tile([128, S], BF16)      # V^T staging for the DMA transpose
            V_sb = pp.tile([128, NT, 256], BF16)  # [V_A |ones| V_B |ones]
            ot_u0 = pp.tile([64, SU], BF16)
            ot_u1 = pp.tile([64, SU], BF16)
            ot_u2 = pp.tile([64, SU], BF16)
            ot_tiles = {"u0": ot_u0, "u1": ot_u1, "u2": ot_u2}
            den_sb = pp.tile([128, 12 * 512], F32)  # rows 64:128 used

            # DMA split: gpsimd + scalar queues are the fast (hardware-
            # dynamic) paths — they carry ALL bulk input, interleaved so the
            # just-in-time order (wq, x0, wk, x1, wv, x2, ...) lands first.
            # The slow sync queue carries only transposes/rlo/output halves.
            # First two chunks split in halves across both queues so the
            # sweep-0 pipeline can start sooner.
            HX = PO * 256
            nc.gpsimd.dma_start(wq_sb[:], wq.ap())
            nc.scalar.dma_start(xT_sb[:, 0, HX : 2 * HX], xT.ap()[:, 0, HX : 2 * HX])
            nc.gpsimd.dma_start(xT_sb[:, 0, 0:HX], xT.ap()[:, 0, 0:HX])
            nc.scalar.dma_start(wk_sb[:], wk.ap())
            nc.gpsimd.dma_start(xT_sb[:, 1, 0:HX], xT.ap()[:, 1, 0:HX])
            nc.scalar.dma_start(xT_sb[:, 1, HX : 2 * HX], xT.ap()[:, 1, HX : 2 * HX])
            nc.gpsimd.dma_start(wv_sb[:], wv.ap())
            # remaining chunks: x2 split across both fast queues, rest
            # alternating (sync must stay clear for the V transposes)
            nc.scalar.dma_start(xT_sb[:, 2, HX : 2 * HX], xT.ap()[:, 2, HX : 2 * HX])
            nc.gpsimd.dma_start(xT_sb[:, 2, 0:HX], xT.ap()[:, 2, 0:HX])
            nc.scalar.dma_start(xT_sb[:, 3], xT.ap()[:, 3])
            nc.gpsimd.dma_start(xT_sb[:, 4], xT.ap()[:, 4])
            nc.scalar.dma_start(xT_sb[:, 5], xT.ap()[:, 5])
            nc.gpsimd.dma_start(xT_sb[:, 6], xT.ap()[:, 6])
            nc.scalar.dma_start(xT_sb[:, 7], xT.ap()[:, 7])
            nc.gpsimd.dma_start(wp_sb[:], wp.ap())

            q0_sb = pp.tile([128, 1], F32)
            bias_sb = pp.tile([128, 1], F32)
            nc.vector.memset(V_sb[:, :, 64:128], 1.0)
            nc.vector.memset(V_sb[:, :, 192:256], 1.0)
            # HAM pre-warm: dense matmuls while the input DMAs stream so the
            # PE clock is at 2.4 GHz when real work starts
            warm_sb = pp.tile([128, 128], BF16)
            nc.vector.memset(warm_sb[:], 0.0)
            warm_ps = psp.tile([128, 512], F32, tag="aux", name="warm_ps")
            for i in range(44):
                nc.tensor.matmul(
                    warm_ps[:, 0:128], lhsT=warm_sb[:], rhs=warm_sb[:],
                    start=(i == 0), stop=(i == 43),
                )
            nc.vector.memset(q0_sb[:], EXP_Q0)
            nc.vector.memset(bias_sb[:], EXP_BIAS)
            expb = _expb_op()

            # ---------------- q/k/v projection blocks -----------------
            pfx_alt = [0]

            def pfx_copy(dst, srcv):
                pfx_alt[0] ^= 1
                if pfx_alt[0]:
                    nc.vector.tensor_copy(dst, srcv)
                else:
                    nc.scalar.copy(dst, srcv)

            def emit_qt_pair(n):
                # n<4: heads A+B queries for LOCAL block n in one full-width
                # chain (bh=1 cores get their x chunks rolled by 4 host-side:
                # softmax over keys is order-invariant and the host
                # un-permutes the output blocks).  n>=4: head A only, blocks n
                # and n+1 as two column-tiled concurrent chains sharing the
                # bank (each chain has its own start/stop; the pending-zero
                # clear is per partition range).
                q_ps = psp.tile([128, 512], F32, tag="aux", name="q_ps")
                for po in range(PO):
                    if n < 4:
                        nc.tensor.matmul(
                            q_ps[:],
                            lhsT=wq_sb[:, ts(po, 128)],
                            rhs=xT_sb[:, n, ts(po, 512)],
                            start=(po == 0),
                            stop=(po == PO - 1),
                        )
                    else:
                        nc.tensor.matmul(
                            q_ps[0:64, :],
                            lhsT=wq_sb[:, ds(po * 128, 64)],
                            rhs=xT_sb[:, n, ts(po, 512)],
                            start=(po == 0),
                            stop=(po == PO - 1),
                            skip_group_check=True,
                        )
                        nc.tensor.matmul(
                            q_ps[64:128, :],
                            lhsT=wq_sb[:, ds(po * 128, 64)],
                            rhs=xT_sb[:, n + 1, ts(po, 512)],
                            start=(po == 0),
                            stop=(po == PO - 1),
                            skip_group_check=True,
                        )
                if n < 4:
                    pfx_copy(QT_sb[:, ts(n, 512)], q_ps[:])
                else:
                    pfx_copy(QT_sb[0:64, ts(n, 512)], q_ps[0:64, :])
                    pfx_copy(QT_sb[0:64, ts(n + 1, 512)], q_ps[64:128, :])

            def emit_kt_block(n):
                k_ps = psp.tile([128, 512], F32, tag="aux", name="k_ps")
                for po in range(PO):
                    nc.tensor.matmul(
                        k_ps[:, 0:512],
                        lhsT=wk_sb[:, ts(po, 128)],
                        rhs=xT_sb[:, n, ts(po, 512)],
                        start=(po == 0),
                        stop=(po == PO - 1),
                    )
                pfx_copy(KT_sb[:, ts(n, 512)], k_ps[:, 0:512])

            def emit_vt_block(n):
                v_ps = psp.tile([128, 512], F32, tag="aux", name="v_ps")
                for po in range(PO):
                    nc.tensor.matmul(
                        v_ps[:, 0:512],
                        lhsT=wv_sb[:, ts(po, 128)],
                        rhs=xT_sb[:, n, ts(po, 512)],
                        start=(po == 0),
                        stop=(po == PO - 1),
                    )
                pfx_copy(VT_sb[:, ts(n, 512)], v_ps[:, 0:512])
                vv = V_sb[:, 4 * n : 4 * n + 4, :]
                nc.sync.dma_start_transpose(vv[:, :, 0:64], VT_sb[0:64, ts(n, 512)])
                nc.sync.dma_start_transpose(vv[:, :, 128:192], VT_sb[64:128, ts(n, 512)])

            # ---------------- attention sweeps ----------------
            # unit specs: (vbase, wp_idx, ydst, ycolbase)
            USPEC = {
                "u0": (0, 0, yTa, 0),
                "u1": (0, 0, yTa, SU),
                "u2": (128, 1, yTb, 0),
            }
            # sweeps: (L, R) sides: (unit, kt_tile, slot, qt_abs_col, ot_local_col)
            sweeps = []
            for sb in range(4):
                sweeps.append(
                    (
                        ("u0", KT_sb, 0, sb * 512, sb * 512),
                        ("u2", KT_sb, 64, sb * 512, sb * 512),
                    )
                )
            for sb in range(2):
                sweeps.append(
                    (
                        ("u1", KT_sb, 0, SU + sb * 512, sb * 512),
                        ("u1", KT2_sb, 64, SU + 1024 + sb * 512, 1024 + sb * 512),
                    )
                )

            proj_q = []
            drain_q = []
            cp_alt = [0]
            oq_alt = [0]
            pj_alt = [0]
            dq_alt = [0]
            rhi_last = [None]

            tw_holder = []
            yst_holder = [None]

            def emit_proj_chunk(u, oe, col, tail=False):
                _vb, wpi, ydst, ybase = USPEC[u]
                # in the tail the score banks are idle: rotate pj through four
                # banks (aux + scL) so the MM->copy loop pipelines deeper
                if tail:
                    pj_alt[0] ^= 1
                    tag = "aux" if pj_alt[0] else "scL"
                else:
                    tag = "aux"
                pj = psp.tile([128, 512], F32, tag=tag, name="pj")
                nc.tensor.matmul(
                    pj[:],
                    lhsT=wp_sb[:, ds(wpi * D + oe * 128, 128)],
                    rhs=ot_tiles[u][0:64, ds(col, 512)],
                    start=True,
                    stop=True,
                )
                if oe == 0:
                    yst_holder[0] = wkp.tile(
                        [128, PO * 512], BF16, tag="yst", name="yst", bufs=2
                    )
                yst = yst_holder[0]
                # PSUM->SBUF copies lean 2:1 on ScalarE: proj copies on DVE
                # delay the exps that gate the R-side scores (measured worse
                # at 50:50 despite the raw engine-time arithmetic)
                if tail:
                    eng = nc.scalar if oe % 2 == 0 else nc.vector
                else:
                    oq_alt[0] = (oq_alt[0] + 1) % 3
                    eng = nc.vector if oq_alt[0] == 0 else nc.scalar
                if eng is nc.scalar:
                    nc.scalar.copy(yst[:, ds(oe * 512, 512)], pj[:])
                else:
                    nc.vector.tensor_copy(yst[:, ds(oe * 512, 512)], pj[:])
                if oe == PO - 1:
                    # contiguous output DMA per (unit, column-block) group,
                    # split in halves across both queues for 2x drain rate
                    jc = (col + ybase) // 512
                    half = PO * 512 // 2
                    nc.sync.dma_start(
                        ydst.ap()[:, jc, 0:half], yst[:, 0:half]
                    )
                    nc.gpsimd.dma_start(
                        ydst.ap()[:, jc, half : 2 * half], yst[:, half : 2 * half]
                    )

            def pump(k):
                for _ in range(k):
                    if proj_q:
                        proj_q.pop(0)()

            # pre-sweep: block-0 projections (just-in-time minimum)
            emit_qt_pair(0)
            emit_kt_block(0)
            emit_vt_block(0)

            for si, (Lside, Rside) in enumerate(sweeps):
                if si == 1:
                    # duplicate for head-A self-pairing (after KT complete)
                    nc.gpsimd.dma_start(KT2_sb[64:128, :], KT_sb[0:64, :])
                elif si == 2:
                    # Q duplicate (after the sweep-1-woven qt(6|7) completes)
                    nc.gpsimd.dma_start(
                        QT_sb[64:128, SU + 1024 : S], QT_sb[0:64, SU + 1024 : S]
                    )
                avL = psp.tile([128, 512], F32, tag="av", name="avL")
                avR = psp.tile([128, 512], F32, tag="av", name="avR")

                def emit_av_pair(p0, p1):
                    # same-bank runs: avL(t0), avL(t1), avR(t0), avR(t1) —
                    # consecutive accumulations into one PSUM bank issue
                    # back-to-back with no bank-switch micro-idle
                    for idx, (u, av) in enumerate(((Lside[0], avL), (Rside[0], avR))):
                        vbase = USPEC[u][0]
                        for t, ptl, ptr in (p0, p1):
                            pt = ptl if idx == 0 else ptr
                            nc.tensor.matmul(
                                av,
                                lhsT=V_sb[:, t, vbase : vbase + 128],
                                rhs=pt[:, 0:512],
                                start=(t == 0),
                                stop=(t == NT - 1),
                            )

                pend = []
                for t in range(NT):
                    if t == 1 and drain_q:
                        # prior sweep's drain emits here so its recip/mul ops
                        # queue BEHIND this sweep's first exps on DVE (the sc
                        # bank recycle path stays low-latency at sweep start)
                        drain_q.pop(0)()
                    # qkv generation woven just-in-time into sweep 0: K block
                    # m+1 lands 2 chunks before its scores need it, so a DMA
                    # wait parks the PE queue as late as possible
                    if si == 0:
                        m = t // 4
                        if t % 4 == 1 and m < 7:
                            emit_kt_block(m + 1)
                        elif t % 4 == 2 and m < 7:
                            emit_vt_block(m + 1)
                        elif t == 4:
                            emit_qt_pair(1)
                    elif si == 1:
                        if t == 0:
                            emit_qt_pair(2)
                        elif t == 4:
                            emit_qt_pair(3)
                        elif t == 8:
                            emit_qt_pair(4)
                        elif t == 12:
                            emit_qt_pair(6)
                    # sweeps >=2 have no qkv weaves, so the aux banks are only
                    # lightly used (12 proj chunks): borrow them for odd-t
                    # L-scores, deepening the scL rotation to ~4 chunks.  The
                    # score then waits exp(t-4) instead of exp(t-2), which
                    # absorbs the exp-queue jitter that was stalling the PE.
                    if si >= 2 and t % 2 == 1:
                        scL = psp.tile([128, 512], F32, tag="aux", name="scLx")
                    else:
                        scL = psp.tile([128, 512], F32, tag="scL", name="scL")
                    scR = psp.tile([128, 512], F32, tag="scR", name="scR")
                    for (u, ktt, slot, qcol, _ocol), sct in ((Lside, scL), (Rside, scR)):
                        nc.tensor.matmul(
                            sct[:, 0:512],
                            lhsT=ktt[slot : slot + 64, ts(t, 128)],
                            rhs=QT_sb[slot : slot + 64, ds(qcol, 512)],
                            start=True,
                            stop=True,
                        )
                    ptL = wkp.tile([128, 512], BF16, tag="ptL", name="ptL", bufs=8)
                    ptR = wkp.tile([128, 512], BF16, tag="ptR", name="ptR", bufs=8)
                    # no bias: the 2^0.5 factor vs the DVE branch's window is a
                    # per-unit global scale that cancels in the normalization
                    nc.scalar.activation(
                        ptL[:, 0:512],
                        scL[:, 0:512],
                        mybir.ActivationFunctionType.Exp,
                        scale=EXP_SCALE,
                    )
                    nc.vector._custom_dve(
                        expb,
                        out=ptR[:, 0:512].bitcast(mybir.dt.uint16),
                        in0=scR[:, 0:512],
                        in1=q0_sb[:],
                        s0=EXP_M,
                        s1=EXP_Q2,
                        imm2=EXP_Q1,
                    )
                    pend.append((t, ptL, ptR))
                    if t % 2 == 1:
                        # AVs lag two chunk-pairs behind the scores so their
                        # exp inputs are done at issue: every PE instruction is
                        # dep-free (throughput-bound, not exp-latency-bound)
                        if len(pend) >= 6:
                            emit_av_pair(pend.pop(0), pend.pop(0))
                        pump(1)
                while len(pend) >= 2:
                    emit_av_pair(pend.pop(0), pend.pop(0))

                # drain: stage UNNORMALIZED O^T + the denominator row; the
                # host divides (softmax normalization is a per-query scalar
                # that commutes with the projection).  Deferred into the NEXT
                # sweep (t==1) so these copies don't block its exps.
                def emit_drain(sides=(Lside, Rside), avs=(avL, avR)):
                    for (u, _ktt, _slot, _qcol, ocol), av in zip(sides, avs):
                        _vb, _wpi, _yd, ybase = USPEC[u]
                        jc = (ocol + ybase) // 512 + (8 if u == "u2" else 0)
                        cp_alt[0] ^= 1
                        e1 = nc.scalar if cp_alt[0] else nc.vector
                        e2 = nc.vector if cp_alt[0] else nc.scalar
                        if e1 is nc.scalar:
                            nc.scalar.copy(
                                ot_tiles[u][0:64, ds(ocol, 512)], av[0:64, :]
                            )
                        else:
                            nc.vector.tensor_copy(
                                ot_tiles[u][0:64, ds(ocol, 512)], av[0:64, :]
                            )
                        if e2 is nc.scalar:
                            nc.scalar.copy(
                                den_sb[64:128, ds(jc * 512, 512)], av[64:128, :]
                            )
                        else:
                            nc.vector.tensor_copy(
                                den_sb[64:128, ds(jc * 512, 512)], av[64:128, :]
                            )
                        for oe in range(PO):
                            proj_q.append(
                                lambda tail=False, u=u, oe=oe, col=ocol: (
                                    emit_proj_chunk(u, oe, col, tail)
                                )
                            )

                drain_q.append(emit_drain)
            # last sweep's deferred drain
            while drain_q:
                drain_q.pop(0)()
            # ship the denominator row (one partition's worth)
            nc.sync.dma_start(den.ap(), den_sb[64:65, :])
            # tail: bridge the last sweep's drain latency with filler matmuls
            # that DEPEND on the drain's output (otherwise the scheduler
            # hoists them ahead of the drain and they fill nothing)
            tw_holder.append(psp.tile([128, 512], F32, tag="scR", name="tw"))
            for _ in range(10):
                nc.tensor.matmul(
                    tw_holder[0][:, 0:448],
                    lhsT=ot_u1[0:64, 0:128],
                    rhs=ot_u1[0:64, 1536:1984],
                    start=True, stop=True,
                )
            while proj_q:
                proj_q.pop(0)(tail=True)

    nc.compile()
    return nc


def _build():
    if "nc" not in _CACHE:
        nc = bacc.Bacc(None, target_bir_lowering=False, debug=False)
        _CACHE["nc"] = _emit(nc)
    return _CACHE["nc"]


def _prep_inputs(x, w_qkv, w_proj):
    bf = ml_dtypes.bfloat16
    xs = np.ascontiguousarray(x.reshape(S, D).T).astype(bf)  # [D, S]
    # chunk-major [pi, n, po*512+s]: element = xT[po*128+pi, n*512+s]
    x8 = np.ascontiguousarray(
        xs.reshape(PO, 128, NC8, 512).transpose(1, 2, 0, 3).reshape(128, NC8, PO * 512)
    )
    in_maps = []
    for c in range(NCORES):
        ha = c
        hb = 8 + c // 2
        bh = c % 2
        rows_q = lambda h: w_qkv[h * HD : (h + 1) * HD, :]
        rows_k = lambda h: w_qkv[D + h * HD : D + (h + 1) * HD, :]
        rows_v = lambda h: w_qkv[2 * D + h * HD : 2 * D + (h + 1) * HD, :]
        qs = SCALE * LOG2E_128
        wq_c = np.concatenate([rows_q(ha) * qs, rows_q(hb) * qs], 0).T  # [D, 128]
        wk_c = np.concatenate([rows_k(ha), rows_k(hb)], 0).T
        wv_c = np.concatenate([rows_v(ha), rows_v(hb)], 0).T
        wp_c = np.stack(
            [w_proj[:, ha * HD : (ha + 1) * HD].T, w_proj[:, hb * HD : (hb + 1) * HD].T],
            axis=1,
        )  # [64, 2, D]
        shuf = lambda w: np.ascontiguousarray(
            w.reshape(PO, 128, 128).transpose(1, 0, 2).reshape(128, PO * 128)
        ).astype(bf)
        # local chunk j = global chunk (j + 4*bh) % 8, so head-B's query rows
        # are always local chunks 0-3 (key order is irrelevant to softmax)
        x8c = np.roll(x8, -4 * bh, axis=1) if bh else x8
        in_maps.append(
            {
                "xT": np.ascontiguousarray(x8c),
                "wq": shuf(wq_c),
                "wk": shuf(wk_c),
                "wv": shuf(wv_c),
                "wp": np.ascontiguousarray(wp_c.reshape(64, 2 * D)).astype(bf),
            }
        )
    return in_maps


def _combine(results, b_proj):
    yT = np.zeros((D, S), np.float32)
    for c in range(NCORES):
        bh = c % 2
        dn = results[c]["den"].astype(np.float32).reshape(12, 512)
        ya = results[c]["yTa"].astype(np.float32)  # [128, 8, 6*512] local order
        ya = ya.reshape(128, NC8, PO, 512) / dn[None, 0:8, None, :]
        if bh:
            ya = np.roll(ya, 4 * bh, axis=1)  # local block j -> global (j+4)%8
        yT += ya.transpose(2, 0, 1, 3).reshape(D, S)
        yb = results[c]["yTb"].astype(np.float32)  # [128, 4, 6*512]
        yb = yb.reshape(128, 4, PO, 512) / dn[None, 8:12, None, :]
        yT[:, bh * SU : (bh + 1) * SU] += yb.transpose(2, 0, 1, 3).reshape(D, SU)
    y = yT.T + b_proj.astype(np.float32)[None, :]
    return y.reshape(1, 64, 64, D).astype(np.float32)


def kernel(x, w_qkv, w_proj, b_proj, _trace=False, _trace_kwargs=None):
    x = np.asarray(x, np.float32)
    w_qkv = np.asarray(w_qkv, np.float32)
    w_proj = np.asarray(w_proj, np.float32)
    b_proj = np.asarray(b_proj, np.float32)

    nc = _build()
    in_maps = _prep_inputs(x, w_qkv, w_proj)
    res = run_bass_kernel_spmd(
        nc, in_maps, core_ids=list(range(NCORES)), trace=_trace,
        **(_trace_kwargs or {}),
    )
    out = _combine(res.results, b_proj)
    if _trace:
        return out, res
    return out


# revision 95
# speedup vs baseline: 1.0512x; 1.0350x over previous
"""Multi-head attention (B=1, S=4096, D=768, 12 heads) on 8 trn2 cores.

Sharding: tensor-parallel by heads, balanced with sequence splits.
Core c owns: head A = c (all 4096 query rows) and head B = 8 + c//2
(query-row half c%2).  Each core computes q/k/v for its two heads, full
S x S attention for its share, and its heads' partial contribution to
the output projection (row-parallel split of w_proj).  The host sums
the per-core projection partials and adds the bias.

Device layout: d-on-partitions ("transposed") everywhere.  Scores are
computed as S^T[t, s] = K^T.T @ Q^T per 128-key chunk; exp split across
ScalarE + the custom DVE op (logits are bounded, no max subtraction);
the AV matmul uses a [V | ones] stationary operand so rows 0-63 of PSUM
accumulate the UNNORMALIZED O^T and rows 64-127 the softmax denominator
in the same pass.  Normalization (a per-query scalar that commutes with
the projection) happens on the HOST: the kernel ships unnormalized
projection partials plus the denominator rows.

The attention loop runs "sweeps" that each process TWO units at once -
one on PE row-groups 0-1 (SBUF partitions 0-63) and one on row-groups
2-3 (partitions 64-127) - so the K=64 score matmuls run pairwise
concurrent on the PE array.  Head A's second half is paired with itself
via a partition-shifted duplicate of Q^T/K^T.  The inner loop is
batched in pairs of key-chunks so the AV accumulations issue as
same-PSUM-bank runs (avoids the per-matmul bank-switch micro-idle), and
each sweep's drain is emitted inside the next sweep so it never blocks
the exp pipeline.  Head B's query rows are always LOCAL x chunks 0-3:
odd cores get their chunks rolled by 4 host-side (key order is
irrelevant to softmax; the host un-permutes the output blocks), which
removes any duplicate query input.  All DMA is chunk-major fully
contiguous (6KB/partition elements), spread across the gpsimd + scalar
hardware queues just-in-time against the sweep-0 weave, with the slow
sync queue reserved for V transposes and output halves.  All matmuls
are bf16 with fp32 PSUM accumulation.
"""

import numpy as np
import ml_dtypes

import concourse.bass as bass
import concourse.mybir as mybir
import concourse.tile as tile
from concourse import bacc
from concourse.bass_utils import run_bass_kernel_spmd

BF16 = mybir.dt.bfloat16
F32 = mybir.dt.float32
ts = bass.ts
ds = bass.ds

S = 4096
D = 768
NH = 12
HD = 64
NCORES = 8
SU = 2048          # rows per unit
PO = D // 128      # 6 e-chunks
NT = S // 128      # 32 key chunks
NC8 = S // 512     # 8 column chunks
SCALE = HD ** -0.5

_CACHE: dict = {}

# --- custom DVE exp op: out_uint16 = bf16 bits of 2^((x - 64)/128) ---------
# Magic-constant round to the 128-grid + quadratic mantissa correction,
# emitted through the fp32->uint16 value cast.  The -64 window shift (a
# global 2^-0.5 factor on all exp values) cancels in the softmax
# normalization; the ScalarE branch matches it via the activation bias.
EXP_M = 1.5 * 2**30
EXP_Q0 = 16180.991964579287
EXP_Q1 = 0.9950478871994926
EXP_Q2 = 0.0026875086476569427
EXP_SCALE = float(np.log(2) / 128.0)
EXP_BIAS = float(-np.log(2) / 2.0)
LOG2E_128 = float(128.0 / np.log(2))


def _expb_ref(in0, in1, s0, s1, imm2):
    f32 = np.float32
    a = (in0.astype(f32) + f32(s0)).astype(f32)
    u = (a - f32(s0)).astype(f32)
    z = (in0.astype(f32) - u).astype(f32)
    m2 = (((z * f32(s1)).astype(f32) + f32(imm2)).astype(f32) * z).astype(f32)
    return ((u + m2).astype(f32) + in1.astype(f32)).astype(f32)


def _expb_op():
    from concourse import dve_ops
    from concourse.dve_spec import Spec, Src0, C0, C1, C2, C3, lower, _spill_c3_to_src1
    from concourse.dve_uop import DveOpSpec

    for op in dve_ops.OPS:
        if op.name == "EXPB_ANT":
            return op
    a = Src0 + C0
    u = a - C0
    z = Src0 - u
    m2 = (z * C1 + C2) * z
    body = _spill_c3_to_src1((u + m2) + C3)
    spec = Spec(body=body, reference=_expb_ref)
    row = dve_ops._CUSTOM_DVE_ROW_BASE + len(dve_ops.OPS)
    dve_ops._SUB_OPCODE_FOR_NAME["EXPB_ANT"] = row
    shas = {}
    for ver in ("v3", "v4"):
        try:
            uops = lower(spec, ver=ver)
            shas[ver] = DveOpSpec(
                name="EXPB_ANT", opcode=row, uops=uops, rd1_en=True
            ).sha(ver)
        except Exception:
            pass
    op = dve_ops.DveOp("EXPB_ANT", spec, subdim=False, uops_sha=shas)
    dve_ops.OPS.append(op)
    dve_ops.CUSTOM_DVE_SPECS["EXPB_ANT"] = spec
    return op


def _emit(nc):
    # chunk-major inputs, flattened so every DMA element is one fully
    # contiguous 6KB-per-partition block (1KB elements run ~5x slower)
    xT = nc.dram_tensor("xT", [128, NC8, PO * 512], BF16, kind="ExternalInput")
    wq = nc.dram_tensor("wq", [128, PO * 128], BF16, kind="ExternalInput")
    wk = nc.dram_tensor("wk", [128, PO * 128], BF16, kind="ExternalInput")
    wv = nc.dram_tensor("wv", [128, PO * 128], BF16, kind="ExternalInput")
    wp = nc.dram_tensor("wp", [64, 2 * D], BF16, kind="ExternalInput")
    yTa = nc.dram_tensor("yTa", [128, NC8, PO * 512], BF16, kind="ExternalOutput")
    yTb = nc.dram_tensor("yTb", [128, 4, PO * 512], BF16, kind="ExternalOutput")
    # per-query softmax denominators (12 column blocks); host divides
    den = nc.dram_tensor("den", [1, 12 * 512], F32, kind="ExternalOutput")

    with tile.TileContext(nc) as tc:
        with (
            tc.tile_pool(name="persist", bufs=1) as pp,
            tc.tile_pool(name="work", bufs=4) as wkp,
            tc.tile_pool(name="ps", bufs=2, space="PSUM") as psp,
        ):
            # ---------------- persistent SBUF tensors ----------------
            xT_sb = pp.tile([128, NC8, PO * 512], BF16)
            wq_sb = pp.tile([128, PO * 128], BF16)
            wk_sb = pp.tile([128, PO * 128], BF16)
            wv_sb = pp.tile([128, PO * 128], BF16)
            wp_sb = pp.tile([64, 2 * D], BF16)
            QT_sb = pp.tile([128, S], BF16)      # 0:64 A (full S); 64:128 B (cols 0:SU) + A-dup (cols 3072:4096)
            KT_sb = pp.tile([128, S], BF16)      # 0:64 A, 64:128 B
            KT2_sb = pp.tile([128, S], BF16)     # 64:128 = copy of A rows (for self-pairing)
            VT_sb = pp.tile([128, S], BF16)      # V^T staging for the DMA transpose
            V_sb = pp.tile([128, NT, 256], BF16)  # [V_A |ones| V_B |ones]
            ot_u0 = pp.tile([64, SU], BF16)
            ot_u1 = pp.tile([64, SU], BF16)
            ot_u2 = pp.tile([64, SU], BF16)
            ot_tiles = {"u0": ot_u0, "u1": ot_u1, "u2": ot_u2}
            den_sb = pp.tile([128, 12 * 512], F32)  # rows 64:128 used

            # DMA split: gpsimd + scalar queues are the fast (hardware-
            # dynamic) paths — they carry ALL bulk input, interleaved so the
            # just-in-time order (wq, x0, wk, x1, wv, x2, ...) lands first.
            # The slow sync queue carries only transposes/rlo/output halves.
            # First two chunks split in halves across both queues so the
            # sweep-0 pipeline can start sooner.
            HX = PO * 256
            nc.gpsimd.dma_start(wq_sb[:], wq.ap())
            nc.scalar.dma_start(xT_sb[:, 0, HX : 2 * HX], xT.ap()[:, 0, HX : 2 * HX])
            nc.gpsimd.dma_start(xT_sb[:, 0, 0:HX], xT.ap()[:, 0, 0:HX])
            nc.scalar.dma_start(wk_sb[:], wk.ap())
            nc.gpsimd.dma_start(xT_sb[:, 1, 0:HX], xT.ap()[:, 1, 0:HX])
            nc.scalar.dma_start(xT_sb[:, 1, HX : 2 * HX], xT.ap()[:, 1, HX : 2 * HX])
            nc.gpsimd.dma_start(wv_sb[:], wv.ap())
            # remaining chunks: x2 split across both fast queues, rest
            # alternating (sync must stay clear for the V transposes)
            nc.scalar.dma_start(xT_sb[:, 2, HX : 2 * HX], xT.ap()[:, 2, HX : 2 * HX])
            nc.gpsimd.dma_start(xT_sb[:, 2, 0:HX], xT.ap()[:, 2, 0:HX])
            nc.scalar.dma_start(xT_sb[:, 3], xT.ap()[:, 3])
            nc.gpsimd.dma_start(xT_sb[:, 4], xT.ap()[:, 4])
            nc.scalar.dma_start(xT_sb[:, 5], xT.ap()[:, 5])
            nc.gpsimd.dma_start(xT_sb[:, 6], xT.ap()[:, 6])
            nc.scalar.dma_start(xT_sb[:, 7], xT.ap()[:, 7])
            nc.gpsimd.dma_start(wp_sb[:], wp.ap())

            q0_sb = pp.tile([128, 1], F32)
            bias_sb = pp.tile([128, 1], F32)
            nc.vector.memset(V_sb[:, :, 64:128], 1.0)
            nc.vector.memset(V_sb[:, :, 192:256], 1.0)
            # HAM pre-warm: dense matmuls while the input DMAs stream so the
            # PE clock is at 2.4 GHz when real work starts
            warm_sb = pp.tile([128, 128], BF16)
            nc.vector.memset(warm_sb[:], 0.0)
            warm_ps = psp.tile([128, 512], F32, tag="aux", name="warm_ps")
            for i in range(44):
                nc.tensor.matmul(
                    warm_ps[:, 0:128], lhsT=warm_sb[:], rhs=warm_sb[:],
                    start=(i == 0), stop=(i == 43),
                )
            nc.vector.memset(q0_sb[:], EXP_Q0)
            nc.vector.memset(bias_sb[:], EXP_BIAS)
            expb = _expb_op()

            # ---------------- q/k/v projection blocks -----------------
            pfx_alt = [0]

            def pfx_copy(dst, srcv):
                pfx_alt[0] ^= 1
                if pfx_alt[0]:
                    nc.vector.tensor_copy(dst, srcv)
                else:
                    nc.scalar.copy(dst, srcv)

            def emit_qt_pair(n):
                # n<4: heads A+B queries for LOCAL block n in one full-width
                # chain (bh=1 cores get their x chunks rolled by 4 host-side:
                # softmax over keys is order-invariant and the host
                # un-permutes the output blocks).  n>=4: head A only, blocks n
                # and n+1 as two column-tiled concurrent chains sharing the
                # bank (each chain has its own start/stop; the pending-zero
                # clear is per partition range).
                q_ps = psp.tile([128, 512], F32, tag="aux", name="q_ps")
                for po in range(PO):
                    if n < 4:
                        nc.tensor.matmul(
                            q_ps[:],
                            lhsT=wq_sb[:, ts(po, 128)],
                            rhs=xT_sb[:, n, ts(po, 512)],
                            start=(po == 0),
                            stop=(po == PO - 1),
                        )
                    else:
                        nc.tensor.matmul(
                            q_ps[0:64, :],
                            lhsT=wq_sb[:, ds(po * 128, 64)],
                            rhs=xT_sb[:, n, ts(po, 512)],
                            start=(po == 0),
                            stop=(po == PO - 1),
                            skip_group_check=True,
                        )
                        nc.tensor.matmul(
                            q_ps[64:128, :],
                            lhsT=wq_sb[:, ds(po * 128, 64)],
                            rhs=xT_sb[:, n + 1, ts(po, 512)],
                            start=(po == 0),
                            stop=(po == PO - 1),
                            skip_group_check=True,
                        )
                if n < 4:
                    pfx_copy(QT_sb[:, ts(n, 512)], q_ps[:])
                else:
                    pfx_copy(QT_sb[0:64, ts(n, 512)], q_ps[0:64, :])
                    pfx_copy(QT_sb[0:64, ts(n + 1, 512)], q_ps[64:128, :])

            def emit_kt_block(n):
                k_ps = psp.tile([128, 512], F32, tag="aux", name="k_ps")
                for po in range(PO):
                    nc.tensor.matmul(
                        k_ps[:, 0:512],
                        lhsT=wk_sb[:, ts(po, 128)],
                        rhs=xT_sb[:, n, ts(po, 512)],
                        start=(po == 0),
                        stop=(po == PO - 1),
                    )
                pfx_copy(KT_sb[:, ts(n, 512)], k_ps[:, 0:512])

            def emit_vt_block(n):
                v_ps = psp.tile([128, 512], F32, tag="aux", name="v_ps")
                for po in range(PO):
                    nc.tensor.matmul(
                        v_ps[:, 0:512],
                        lhsT=wv_sb[:, ts(po, 128)],
                        rhs=xT_sb[:, n, ts(po, 512)],
                        start=(po == 0),
                        stop=(po == PO - 1),
                    )
                pfx_copy(VT_sb[:, ts(n, 512)], v_ps[:, 0:512])
                vv = V_sb[:, 4 * n : 4 * n + 4, :]
                nc.sync.dma_start_transpose(vv[:, :, 0:64], VT_sb[0:64, ts(n, 512)])
                nc.sync.dma_start_transpose(vv[:, :, 128:192], VT_sb[64:128, ts(n, 512)])

            # ---------------- attention sweeps ----------------
            # unit specs: (vbase, wp_idx, ydst, ycolbase)
            USPEC = {
                "u0": (0, 0, yTa, 0),
                "u1": (0, 0, yTa, SU),
                "u2": (128, 1, yTb, 0),
            }
            # sweeps: (L, R) sides: (unit, kt_tile, slot, qt_abs_col, ot_local_col)
            sweeps = []
            for sb in range(4):
                sweeps.append(
                    (
                        ("u0", KT_sb, 0, sb * 512, sb * 512),
                        ("u2", KT_sb, 64, sb * 512, sb * 512),
                    )
                )
            for sb in range(2):
                sweeps.append(
                    (
                        ("u1", KT_sb, 0, SU + sb * 512, sb * 512),
                        ("u1", KT2_sb, 64, SU + 1024 + sb * 512, 1024 + sb * 512),
                    )
                )

            proj_q = []
            drain_q = []
            cp_alt = [0]
            oq_alt = [0]
            pj_alt = [0]
            dq_alt = [0]
            rhi_last = [None]

            tw_holder = []
            yst_holder = [None]

            def emit_proj_chunk(u, oe, col, tail=False):
                _vb, wpi, ydst, ybase = USPEC[u]
                # in the tail the score banks are idle: rotate pj through four
                # banks (aux + scL) so the MM->copy loop pipelines deeper
                if tail:
                    pj_alt[0] ^= 1
                    tag = "aux" if pj_alt[0] else "scL"
                else:
                    tag = "aux"
                pj = psp.tile([128, 512], F32, tag=tag, name="pj")
                nc.tensor.matmul(
                    pj[:],
                    lhsT=wp_sb[:, ds(wpi * D + oe * 128, 128)],
                    rhs=ot_tiles[u][0:64, ds(col, 512)],
                    start=True,
                    stop=True,
                )
                if oe == 0:
                    yst_holder[0] = wkp.tile(
                        [128, PO * 512], BF16, tag="yst", name="yst", bufs=2
                    )
                yst = yst_holder[0]
                # PSUM->SBUF copies lean 2:1 on ScalarE: proj copies on DVE
                # delay the exps that gate the R-side scores (measured worse
                # at 50:50 despite the raw engine-time arithmetic)
                if tail:
                    eng = nc.scalar if oe % 2 == 0 else nc.vector
                else:
                    oq_alt[0] = (oq_alt[0] + 1) % 3
                    eng = nc.vector if oq_alt[0] == 0 else nc.scalar
                if eng is nc.scalar:
                    nc.scalar.copy(yst[:, ds(oe * 512, 512)], pj[:])
                else:
                    nc.vector.tensor_copy(yst[:, ds(oe * 512, 512)], pj[:])
                if oe == PO - 1:
                    # contiguous output DMA per (unit, column-block) group,
                    # split in halves across both queues for 2x drain rate
                    jc = (col + ybase) // 512
                    half = PO * 512 // 2
                    nc.sync.dma_start(
                        ydst.ap()[:, jc, 0:half], yst[:, 0:half]
                    )
                    nc.gpsimd.dma_start(
                        ydst.ap()[:, jc, half : 2 * half], yst[:, half : 2 * half]
                    )

            def pump(k):
                for _ in range(k):
                    if proj_q:
                        proj_q.pop(0)()

            # pre-sweep: block-0 projections (just-in-time minimum)
            emit_qt_pair(0)
            emit_kt_block(0)
            emit_vt_block(0)

            for si, (Lside, Rside) in enumerate(sweeps):
                if si == 1:
                    # duplicate for head-A self-pairing (after KT complete)
                    nc.gpsimd.dma_start(KT2_sb[64:128, :], KT_sb[0:64, :])
                elif si == 2:
                    # Q duplicate (after the sweep-1-woven qt(6|7) completes)
                    nc.gpsimd.dma_start(
                        QT_sb[64:128, SU + 1024 : S], QT_sb[0:64, SU + 1024 : S]
                    )
                avL = psp.tile([128, 512], F32, tag="av", name="avL")
                avR = psp.tile([128, 512], F32, tag="av", name="avR")

                def emit_av_pair(p0, p1):
                    # same-bank runs: avL(t0), avL(t1), avR(t0), avR(t1) —
                    # consecutive accumulations into one PSUM bank issue
                    # back-to-back with no bank-switch micro-idle
                    for idx, (u, av) in enumerate(((Lside[0], avL), (Rside[0], avR))):
                        vbase = USPEC[u][0]
                        for t, ptl, ptr in (p0, p1):
                            pt = ptl if idx == 0 else ptr
                            nc.tensor.matmul(
                                av,
                                lhsT=V_sb[:, t, vbase : vbase + 128],
                                rhs=pt[:, 0:512],
                                start=(t == 0),
                                stop=(t == NT - 1),
                            )

                pend = []
                for t in range(NT):
                    if t == 1 and drain_q:
                        # prior sweep's drain emits here so its recip/mul ops
                        # queue BEHIND this sweep's first exps on DVE (the sc
                        # bank recycle path stays low-latency at sweep start)
                        drain_q.pop(0)()
                    # qkv generation woven just-in-time into sweep 0: K block
                    # m+1 lands 2 chunks before its scores need it, so a DMA
                    # wait parks the PE queue as late as possible
                    if si == 0:
                        m = t // 4
                        if t % 4 == 1 and m < 7:
                            emit_kt_block(m + 1)
                        elif t % 4 == 2 and m < 7:
                            emit_vt_block(m + 1)
                        elif t == 4:
                            emit_qt_pair(1)
                    elif si == 1:
                        if t == 0:
                            emit_qt_pair(2)
                        elif t == 4:
                            emit_qt_pair(3)
                        elif t == 8:
                            emit_qt_pair(4)
                        elif t == 12:
                            emit_qt_pair(6)
                    # sweeps >=2 have no qkv weaves, so the aux banks are only
                    # lightly used (12 proj chunks): borrow them for odd-t
                    # L-scores, deepening the scL rotation to ~4 chunks.  The
                    # score then waits exp(t-4) instead of exp(t-2), which
                    # absorbs the exp-queue jitter that was stalling the PE.
                    if si >= 2 and t % 2 == 1:
                        scL = psp.tile([128, 512], F32, tag="aux", name="scLx")
                    else:
                        scL = psp.tile([128, 512], F32, tag="scL", name="scL")
                    scR = psp.tile([128, 512], F32, tag="scR", name="scR")
                    for (u, ktt, slot, qcol, _ocol), sct in ((Lside, scL), (Rside, scR)):
                        nc.tensor.matmul(
                            sct[:, 0:512],
                            lhsT=ktt[slot : slot + 64, ts(t, 128)],
                            rhs=QT_sb[slot : slot + 64, ds(qcol, 512)],
                            start=True,
                            stop=True,
                        )
                    ptL = wkp.tile([128, 512], BF16, tag="ptL", name="ptL", bufs=14)
                    ptR = wkp.tile([128, 512], BF16, tag="ptR", name="ptR", bufs=14)
                    # no bias: the 2^0.5 factor vs the DVE branch's window is a
                    # per-unit global scale that cancels in the normalization
                    nc.scalar.activation(
                        ptL[:, 0:512],
                        scL[:, 0:512],
                        mybir.ActivationFunctionType.Exp,
                        scale=EXP_SCALE,
                    )
                    nc.vector._custom_dve(
                        expb,
                        out=ptR[:, 0:512].bitcast(mybir.dt.uint16),
                        in0=scR[:, 0:512],
                        in1=q0_sb[:],
                        s0=EXP_M,
                        s1=EXP_Q2,
                        imm2=EXP_Q1,
                    )
                    pend.append((t, ptL, ptR))
                    if t % 2 == 1:
                        # AVs lag the scores so their exp inputs are done at
                        # issue: every PE instruction is dep-free (throughput-
                        # bound, not exp-latency-bound).  Sweep 0 lags much
                        # deeper: the first V transposes cannot land before
                        # ~34us (gated behind the bulk input DMAs), so scores
                        # and qkv weaves fill the PE until V is ready instead
                        # of parking the queue on the first AV.
                        if len(pend) >= (12 if si == 0 else 6):
                            emit_av_pair(pend.pop(0), pend.pop(0))
                        pump(1)
                while len(pend) >= 2:
                    emit_av_pair(pend.pop(0), pend.pop(0))

                # drain: stage UNNORMALIZED O^T + the denominator row; the
                # host divides (softmax normalization is a per-query scalar
                # that commutes with the projection).  Deferred into the NEXT
                # sweep (t==1) so these copies don't block its exps.
                def emit_drain(sides=(Lside, Rside), avs=(avL, avR)):
                    for (u, _ktt, _slot, _qcol, ocol), av in zip(sides, avs):
                        _vb, _wpi, _yd, ybase = USPEC[u]
                        jc = (ocol + ybase) // 512 + (8 if u == "u2" else 0)
                        cp_alt[0] ^= 1
                        e1 = nc.scalar if cp_alt[0] else nc.vector
                        e2 = nc.vector if cp_alt[0] else nc.scalar
                        if e1 is nc.scalar:
                            nc.scalar.copy(
                                ot_tiles[u][0:64, ds(ocol, 512)], av[0:64, :]
                            )
                        else:
                            nc.vector.tensor_copy(
                                ot_tiles[u][0:64, ds(ocol, 512)], av[0:64, :]
                            )
                        if e2 is nc.scalar:
                            nc.scalar.copy(
                                den_sb[64:128, ds(jc * 512, 512)], av[64:128, :]
                            )
                        else:
                            nc.vector.tensor_copy(
                                den_sb[64:128, ds(jc * 512, 512)], av[64:128, :]
                            )
                        for oe in range(PO):
                            proj_q.append(
                                lambda tail=False, u=u, oe=oe, col=ocol: (
                                    emit_proj_chunk(u, oe, col, tail)
                                )
                            )

                drain_q.append(emit_drain)
            # last sweep's deferred drain
            while drain_q:
                drain_q.pop(0)()
            # ship the denominator row (one partition's worth)
            nc.sync.dma_start(den.ap(), den_sb[64:65, :])
            # tail: bridge the last sweep's drain latency with filler matmuls
            # that DEPEND on the drain's output (otherwise the scheduler
            # hoists them ahead of the drain and they fill nothing)
            tw_holder.append(psp.tile([128, 512], F32, tag="scR", name="tw"))
            for _ in range(10):
                nc.tensor.matmul(
                    tw_holder[0][:, 0:448],
                    lhsT=ot_u1[0:64, 0:128],
                    rhs=ot_u1[0:64, 1536:1984],
                    start=True, stop=True,
                )
            while proj_q:
                proj_q.pop(0)(tail=True)

    nc.compile()
    return nc


def _build():
    if "nc" not in _CACHE:
        nc = bacc.Bacc(None, target_bir_lowering=False, debug=False)
        _CACHE["nc"] = _emit(nc)
    return _CACHE["nc"]


def _prep_inputs(x, w_qkv, w_proj):
    bf = ml_dtypes.bfloat16
    xs = np.ascontiguousarray(x.reshape(S, D).T).astype(bf)  # [D, S]
    # chunk-major [pi, n, po*512+s]: element = xT[po*128+pi, n*512+s]
    x8 = np.ascontiguousarray(
        xs.reshape(PO, 128, NC8, 512).transpose(1, 2, 0, 3).reshape(128, NC8, PO * 512)
    )
    in_maps = []
    for c in range(NCORES):
        ha = c
        hb = 8 + c // 2
        bh = c % 2
        rows_q = lambda h: w_qkv[h * HD : (h + 1) * HD, :]
        rows_k = lambda h: w_qkv[D + h * HD : D + (h + 1) * HD, :]
        rows_v = lambda h: w_qkv[2 * D + h * HD : 2 * D + (h + 1) * HD, :]
        qs = SCALE * LOG2E_128
        wq_c = np.concatenate([rows_q(ha) * qs, rows_q(hb) * qs], 0).T  # [D, 128]
        wk_c = np.concatenate([rows_k(ha), rows_k(hb)], 0).T
        wv_c = np.concatenate([rows_v(ha), rows_v(hb)], 0).T
        wp_c = np.stack(
            [w_proj[:, ha * HD : (ha + 1) * HD].T, w_proj[:, hb * HD : (hb + 1) * HD].T],
            axis=1,
        )  # [64, 2, D]
        shuf = lambda w: np.ascontiguousarray(
            w.reshape(PO, 128, 128).transpose(1, 0, 2).reshape(128, PO * 128)
        ).astype(bf)
        # local chunk j = global chunk (j + 4*bh) % 8, so head-B's query rows
        # are always local chunks 0-3 (key order is irrelevant to softmax)
        x8c = np.roll(x8, -4 * bh, axis=1) if bh else x8
        in_maps.append(
            {
                "xT": np.ascontiguousarray(x8c),
                "wq": shuf(wq_c),
                "wk": shuf(wk_c),
                "wv": shuf(wv_c),
                "wp": np.ascontiguousarray(wp_c.reshape(64, 2 * D)).astype(bf),
            }
        )
    return in_maps


def _combine(results, b_proj):
    yT = np.zeros((D, S), np.float32)
    for c in range(NCORES):
        bh = c % 2
        dn = results[c]["den"].astype(np.float32).reshape(12, 512)
        ya = results[c]["yTa"].astype(np.float32)  # [128, 8, 6*512] local order
        ya = ya.reshape(128, NC8, PO, 512) / dn[None, 0:8, None, :]
        if bh:
            ya = np.roll(ya, 4 * bh, axis=1)  # local block j -> global (j+4)%8
        yT += ya.transpose(2, 0, 1, 3).reshape(D, S)
        yb = results[c]["yTb"].astype(np.float32)  # [128, 4, 6*512]
        yb = yb.reshape(128, 4, PO, 512) / dn[None, 8:12, None, :]
        yT[:, bh * SU : (bh + 1) * SU] += yb.transpose(2, 0, 1, 3).reshape(D, SU)
    y = yT.T + b_proj.astype(np.float32)[None, :]
    return y.reshape(1, 64, 64, D).astype(np.float32)


def kernel(x, w_qkv, w_proj, b_proj, _trace=False, _trace_kwargs=None):
    x = np.asarray(x, np.float32)
    w_qkv = np.asarray(w_qkv, np.float32)
    w_proj = np.asarray(w_proj, np.float32)
    b_proj = np.asarray(b_proj, np.float32)

    nc = _build()
    in_maps = _prep_inputs(x, w_qkv, w_proj)
    res = run_bass_kernel_spmd(
        nc, in_maps, core_ids=list(range(NCORES)), trace=_trace,
        **(_trace_kwargs or {}),
    )
    out = _combine(res.results, b_proj)
    if _trace:
        return out, res
    return out


# revision 96
# speedup vs baseline: 1.0691x; 1.0170x over previous
"""Multi-head attention (B=1, S=4096, D=768, 12 heads) on 8 trn2 cores.

Sharding: tensor-parallel by heads, balanced with sequence splits.
Core c owns: head A = c (all 4096 query rows) and head B = 8 + c//2
(query-row half c%2).  Each core computes q/k/v for its two heads, full
S x S attention for its share, and its heads' partial contribution to
the output projection (row-parallel split of w_proj).  The host sums
the per-core projection partials and adds the bias.

Device layout: d-on-partitions ("transposed") everywhere.  Scores are
computed as S^T[t, s] = K^T.T @ Q^T per 128-key chunk; exp split across
ScalarE + the custom DVE op (logits are bounded, no max subtraction);
the AV matmul uses a [V | ones] stationary operand so rows 0-63 of PSUM
accumulate the UNNORMALIZED O^T and rows 64-127 the softmax denominator
in the same pass.  Normalization (a per-query scalar that commutes with
the projection) happens on the HOST: the kernel ships unnormalized
projection partials plus the denominator rows.

The attention loop runs "sweeps" that each process TWO units at once -
one on PE row-groups 0-1 (SBUF partitions 0-63) and one on row-groups
2-3 (partitions 64-127) - so the K=64 score matmuls run pairwise
concurrent on the PE array.  Head A's second half is paired with itself
via a partition-shifted duplicate of Q^T/K^T.  The inner loop is
batched in pairs of key-chunks so the AV accumulations issue as
same-PSUM-bank runs (avoids the per-matmul bank-switch micro-idle), and
each sweep's drain is emitted inside the next sweep so it never blocks
the exp pipeline.  Head B's query rows are always LOCAL x chunks 0-3:
odd cores get their chunks rolled by 4 host-side (key order is
irrelevant to softmax; the host un-permutes the output blocks), which
removes any duplicate query input.  All DMA is chunk-major fully
contiguous (6KB/partition elements), spread across the gpsimd + scalar
hardware queues just-in-time against the sweep-0 weave, with the slow
sync queue reserved for V transposes and output halves.  All matmuls
are bf16 with fp32 PSUM accumulation.
"""

import numpy as np
import ml_dtypes

import concourse.bass as bass
import concourse.mybir as mybir
import concourse.tile as tile
from concourse import bacc
from concourse.bass_utils import run_bass_kernel_spmd

BF16 = mybir.dt.bfloat16
F32 = mybir.dt.float32
ts = bass.ts
ds = bass.ds

S = 4096
D = 768
NH = 12
HD = 64
NCORES = 8
SU = 2048          # rows per unit
PO = D // 128      # 6 e-chunks
NT = S // 128      # 32 key chunks
NC8 = S // 512     # 8 column chunks
SCALE = HD ** -0.5

_CACHE: dict = {}

# --- custom DVE exp op: out_uint16 = bf16 bits of 2^((x - 64)/128) ---------
# Magic-constant round to the 128-grid + quadratic mantissa correction,
# emitted through the fp32->uint16 value cast.  The -64 window shift (a
# global 2^-0.5 factor on all exp values) cancels in the softmax
# normalization; the ScalarE branch matches it via the activation bias.
EXP_M = 1.5 * 2**30
EXP_Q0 = 16180.991964579287
EXP_Q1 = 0.9950478871994926
EXP_Q2 = 0.0026875086476569427
EXP_SCALE = float(np.log(2) / 128.0)
EXP_BIAS = float(-np.log(2) / 2.0)
LOG2E_128 = float(128.0 / np.log(2))


def _expb_ref(in0, in1, s0, s1, imm2):
    f32 = np.float32
    a = (in0.astype(f32) + f32(s0)).astype(f32)
    u = (a - f32(s0)).astype(f32)
    z = (in0.astype(f32) - u).astype(f32)
    m2 = (((z * f32(s1)).astype(f32) + f32(imm2)).astype(f32) * z).astype(f32)
    return ((u + m2).astype(f32) + in1.astype(f32)).astype(f32)


def _expb_op():
    from concourse import dve_ops
    from concourse.dve_spec import Spec, Src0, C0, C1, C2, C3, lower, _spill_c3_to_src1
    from concourse.dve_uop import DveOpSpec

    for op in dve_ops.OPS:
        if op.name == "EXPB_ANT":
            return op
    a = Src0 + C0
    u = a - C0
    z = Src0 - u
    m2 = (z * C1 + C2) * z
    body = _spill_c3_to_src1((u + m2) + C3)
    spec = Spec(body=body, reference=_expb_ref)
    row = dve_ops._CUSTOM_DVE_ROW_BASE + len(dve_ops.OPS)
    dve_ops._SUB_OPCODE_FOR_NAME["EXPB_ANT"] = row
    shas = {}
    for ver in ("v3", "v4"):
        try:
            uops = lower(spec, ver=ver)
            shas[ver] = DveOpSpec(
                name="EXPB_ANT", opcode=row, uops=uops, rd1_en=True
            ).sha(ver)
        except Exception:
            pass
    op = dve_ops.DveOp("EXPB_ANT", spec, subdim=False, uops_sha=shas)
    dve_ops.OPS.append(op)
    dve_ops.CUSTOM_DVE_SPECS["EXPB_ANT"] = spec
    return op


def _emit(nc):
    # chunk-major inputs, flattened so every DMA element is one fully
    # contiguous 6KB-per-partition block (1KB elements run ~5x slower)
    xT = nc.dram_tensor("xT", [128, NC8, PO * 512], BF16, kind="ExternalInput")
    wq = nc.dram_tensor("wq", [128, PO * 128], BF16, kind="ExternalInput")
    wk = nc.dram_tensor("wk", [128, PO * 128], BF16, kind="ExternalInput")
    wv = nc.dram_tensor("wv", [128, PO * 128], BF16, kind="ExternalInput")
    wp = nc.dram_tensor("wp", [64, 2 * D], BF16, kind="ExternalInput")
    yTa = nc.dram_tensor("yTa", [128, NC8, PO * 512], BF16, kind="ExternalOutput")
    yTb = nc.dram_tensor("yTb", [128, 4, PO * 512], BF16, kind="ExternalOutput")
    # per-query softmax denominators (12 column blocks); host divides
    den = nc.dram_tensor("den", [1, 12 * 512], F32, kind="ExternalOutput")

    with tile.TileContext(nc) as tc:
        with (
            tc.tile_pool(name="persist", bufs=1) as pp,
            tc.tile_pool(name="work", bufs=4) as wkp,
            tc.tile_pool(name="ps", bufs=2, space="PSUM") as psp,
        ):
            # ---------------- persistent SBUF tensors ----------------
            xT_sb = pp.tile([128, NC8, PO * 512], BF16)
            wq_sb = pp.tile([128, PO * 128], BF16)
            wk_sb = pp.tile([128, PO * 128], BF16)
            wv_sb = pp.tile([128, PO * 128], BF16)
            wp_sb = pp.tile([64, 2 * D], BF16)
            QT_sb = pp.tile([128, S], BF16)      # 0:64 A (full S); 64:128 B (cols 0:SU) + A-dup (cols 3072:4096)
            KT_sb = pp.tile([128, S], BF16)      # 0:64 A, 64:128 B
            KT2_sb = pp.tile([128, S], BF16)     # 64:128 = copy of A rows (for self-pairing)
            VT_sb = pp.tile([128, S], BF16)      # V^T staging for the DMA transpose
            V_sb = pp.tile([128, NT, 256], BF16)  # [V_A |ones| V_B |ones]
            ot_u0 = pp.tile([64, SU], BF16)
            ot_u1 = pp.tile([64, SU], BF16)
            ot_u2 = pp.tile([64, SU], BF16)
            ot_tiles = {"u0": ot_u0, "u1": ot_u1, "u2": ot_u2}
            den_sb = pp.tile([128, 12 * 512], F32)  # rows 64:128 used

            # DMA split: gpsimd + scalar queues are the fast (hardware-
            # dynamic) paths — they carry ALL bulk input, interleaved so the
            # just-in-time order (wq, x0, wk, x1, wv, x2, ...) lands first.
            # The slow sync queue carries only transposes/rlo/output halves.
            # First two chunks split in halves across both queues so the
            # sweep-0 pipeline can start sooner.
            HX = PO * 256
            nc.gpsimd.dma_start(wq_sb[:], wq.ap())
            nc.scalar.dma_start(xT_sb[:, 0, HX : 2 * HX], xT.ap()[:, 0, HX : 2 * HX])
            nc.gpsimd.dma_start(xT_sb[:, 0, 0:HX], xT.ap()[:, 0, 0:HX])
            nc.scalar.dma_start(wk_sb[:], wk.ap())
            nc.gpsimd.dma_start(xT_sb[:, 1, 0:HX], xT.ap()[:, 1, 0:HX])
            nc.scalar.dma_start(xT_sb[:, 1, HX : 2 * HX], xT.ap()[:, 1, HX : 2 * HX])
            nc.gpsimd.dma_start(wv_sb[:], wv.ap())
            # remaining chunks: x2 split across both fast queues, rest
            # alternating (sync must stay clear for the V transposes)
            nc.scalar.dma_start(xT_sb[:, 2, HX : 2 * HX], xT.ap()[:, 2, HX : 2 * HX])
            nc.gpsimd.dma_start(xT_sb[:, 2, 0:HX], xT.ap()[:, 2, 0:HX])
            nc.scalar.dma_start(xT_sb[:, 3], xT.ap()[:, 3])
            nc.gpsimd.dma_start(xT_sb[:, 4], xT.ap()[:, 4])
            nc.scalar.dma_start(xT_sb[:, 5], xT.ap()[:, 5])
            nc.gpsimd.dma_start(xT_sb[:, 6], xT.ap()[:, 6])
            nc.scalar.dma_start(xT_sb[:, 7], xT.ap()[:, 7])
            nc.gpsimd.dma_start(wp_sb[:], wp.ap())

            q0_sb = pp.tile([128, 1], F32)
            bias_sb = pp.tile([128, 1], F32)
            nc.vector.memset(V_sb[:, :, 64:128], 1.0)
            nc.vector.memset(V_sb[:, :, 192:256], 1.0)
            # HAM pre-warm: dense matmuls while the input DMAs stream so the
            # PE clock is at 2.4 GHz when real work starts
            warm_sb = pp.tile([128, 128], BF16)
            nc.vector.memset(warm_sb[:], 0.0)
            warm_ps = psp.tile([128, 512], F32, tag="aux", name="warm_ps")
            for i in range(44):
                nc.tensor.matmul(
                    warm_ps[:, 0:128], lhsT=warm_sb[:], rhs=warm_sb[:],
                    start=(i == 0), stop=(i == 43),
                )
            nc.vector.memset(q0_sb[:], EXP_Q0)
            nc.vector.memset(bias_sb[:], EXP_BIAS)
            expb = _expb_op()

            # ---------------- q/k/v projection blocks -----------------
            pfx_alt = [0]

            def pfx_copy(dst, srcv):
                pfx_alt[0] ^= 1
                if pfx_alt[0]:
                    nc.vector.tensor_copy(dst, srcv)
                else:
                    nc.scalar.copy(dst, srcv)

            def emit_qt_pair(n):
                # n<4: heads A+B queries for LOCAL block n in one full-width
                # chain (bh=1 cores get their x chunks rolled by 4 host-side:
                # softmax over keys is order-invariant and the host
                # un-permutes the output blocks).  n>=4: head A only, blocks n
                # and n+1 as two column-tiled concurrent chains sharing the
                # bank (each chain has its own start/stop; the pending-zero
                # clear is per partition range).
                q_ps = psp.tile([128, 512], F32, tag="aux", name="q_ps")
                for po in range(PO):
                    if n < 4:
                        nc.tensor.matmul(
                            q_ps[:],
                            lhsT=wq_sb[:, ts(po, 128)],
                            rhs=xT_sb[:, n, ts(po, 512)],
                            start=(po == 0),
                            stop=(po == PO - 1),
                        )
                    else:
                        nc.tensor.matmul(
                            q_ps[0:64, :],
                            lhsT=wq_sb[:, ds(po * 128, 64)],
                            rhs=xT_sb[:, n, ts(po, 512)],
                            start=(po == 0),
                            stop=(po == PO - 1),
                            skip_group_check=True,
                        )
                        nc.tensor.matmul(
                            q_ps[64:128, :],
                            lhsT=wq_sb[:, ds(po * 128, 64)],
                            rhs=xT_sb[:, n + 1, ts(po, 512)],
                            start=(po == 0),
                            stop=(po == PO - 1),
                            skip_group_check=True,
                        )
                if n < 4:
                    pfx_copy(QT_sb[:, ts(n, 512)], q_ps[:])
                else:
                    pfx_copy(QT_sb[0:64, ts(n, 512)], q_ps[0:64, :])
                    pfx_copy(QT_sb[0:64, ts(n + 1, 512)], q_ps[64:128, :])

            def emit_kt_block(n):
                k_ps = psp.tile([128, 512], F32, tag="aux", name="k_ps")
                for po in range(PO):
                    nc.tensor.matmul(
                        k_ps[:, 0:512],
                        lhsT=wk_sb[:, ts(po, 128)],
                        rhs=xT_sb[:, n, ts(po, 512)],
                        start=(po == 0),
                        stop=(po == PO - 1),
                    )
                pfx_copy(KT_sb[:, ts(n, 512)], k_ps[:, 0:512])

            def emit_vt_block(n):
                v_ps = psp.tile([128, 512], F32, tag="aux", name="v_ps")
                for po in range(PO):
                    nc.tensor.matmul(
                        v_ps[:, 0:512],
                        lhsT=wv_sb[:, ts(po, 128)],
                        rhs=xT_sb[:, n, ts(po, 512)],
                        start=(po == 0),
                        stop=(po == PO - 1),
                    )
                pfx_copy(VT_sb[:, ts(n, 512)], v_ps[:, 0:512])
                vv = V_sb[:, 4 * n : 4 * n + 4, :]
                nc.sync.dma_start_transpose(vv[:, :, 0:64], VT_sb[0:64, ts(n, 512)])
                nc.sync.dma_start_transpose(vv[:, :, 128:192], VT_sb[64:128, ts(n, 512)])

            # ---------------- attention sweeps ----------------
            # unit specs: (vbase, wp_idx, ydst, ycolbase)
            USPEC = {
                "u0": (0, 0, yTa, 0),
                "u1": (0, 0, yTa, SU),
                "u2": (128, 1, yTb, 0),
            }
            # sweeps: (L, R) sides: (unit, kt_tile, slot, qt_abs_col, ot_local_col)
            sweeps = []
            for sb in range(4):
                sweeps.append(
                    (
                        ("u0", KT_sb, 0, sb * 512, sb * 512),
                        ("u2", KT_sb, 64, sb * 512, sb * 512),
                    )
                )
            for sb in range(2):
                sweeps.append(
                    (
                        ("u1", KT_sb, 0, SU + sb * 512, sb * 512),
                        ("u1", KT2_sb, 64, SU + 1024 + sb * 512, 1024 + sb * 512),
                    )
                )

            proj_q = []
            drain_q = []
            cp_alt = [0]
            oq_alt = [0]
            pj_alt = [0]
            dq_alt = [0]
            rhi_last = [None]

            tw_holder = []
            yst_holder = [None]

            def emit_proj_chunk(u, oe, col, tail=False):
                _vb, wpi, ydst, ybase = USPEC[u]
                # in the tail the score banks are idle: rotate pj through four
                # banks (aux + scL) so the MM->copy loop pipelines deeper
                if tail:
                    pj_alt[0] ^= 1
                    tag = "aux" if pj_alt[0] else "scL"
                else:
                    tag = "aux"
                pj = psp.tile([128, 512], F32, tag=tag, name="pj")
                nc.tensor.matmul(
                    pj[:],
                    lhsT=wp_sb[:, ds(wpi * D + oe * 128, 128)],
                    rhs=ot_tiles[u][0:64, ds(col, 512)],
                    start=True,
                    stop=True,
                )
                if oe == 0:
                    yst_holder[0] = wkp.tile(
                        [128, PO * 512], BF16, tag="yst", name="yst", bufs=2
                    )
                yst = yst_holder[0]
                # PSUM->SBUF copies lean 2:1 on ScalarE: proj copies on DVE
                # delay the exps that gate the R-side scores (measured worse
                # at 50:50 despite the raw engine-time arithmetic)
                if tail:
                    eng = nc.scalar if oe % 2 == 0 else nc.vector
                else:
                    oq_alt[0] = (oq_alt[0] + 1) % 3
                    eng = nc.vector if oq_alt[0] == 0 else nc.scalar
                if eng is nc.scalar:
                    nc.scalar.copy(yst[:, ds(oe * 512, 512)], pj[:])
                else:
                    nc.vector.tensor_copy(yst[:, ds(oe * 512, 512)], pj[:])
                if oe == PO - 1:
                    # contiguous output DMA per (unit, column-block) group,
                    # split in halves across both queues for 2x drain rate
                    jc = (col + ybase) // 512
                    half = PO * 512 // 2
                    nc.sync.dma_start(
                        ydst.ap()[:, jc, 0:half], yst[:, 0:half]
                    )
                    nc.gpsimd.dma_start(
                        ydst.ap()[:, jc, half : 2 * half], yst[:, half : 2 * half]
                    )

            def pump(k):
                for _ in range(k):
                    if proj_q:
                        proj_q.pop(0)()

            # pre-sweep: block-0 projections (just-in-time minimum)
            emit_qt_pair(0)
            emit_kt_block(0)
            emit_vt_block(0)

            for si, (Lside, Rside) in enumerate(sweeps):
                if si == 1:
                    # duplicate for head-A self-pairing (after KT complete)
                    nc.gpsimd.dma_start(KT2_sb[64:128, :], KT_sb[0:64, :])
                elif si == 2:
                    # Q duplicate (after the sweep-1-woven qt(6|7) completes)
                    nc.gpsimd.dma_start(
                        QT_sb[64:128, SU + 1024 : S], QT_sb[0:64, SU + 1024 : S]
                    )
                avL = psp.tile([128, 512], F32, tag="av", name="avL")
                avR = psp.tile([128, 512], F32, tag="av", name="avR")

                def emit_av_pair(p0, p1):
                    # same-bank runs: avL(t0), avL(t1), avR(t0), avR(t1) —
                    # consecutive accumulations into one PSUM bank issue
                    # back-to-back with no bank-switch micro-idle
                    for idx, (u, av) in enumerate(((Lside[0], avL), (Rside[0], avR))):
                        vbase = USPEC[u][0]
                        for t, ptl, ptr in (p0, p1):
                            pt = ptl if idx == 0 else ptr
                            nc.tensor.matmul(
                                av,
                                lhsT=V_sb[:, t, vbase : vbase + 128],
                                rhs=pt[:, 0:512],
                                start=(t == 0),
                                stop=(t == NT - 1),
                            )

                pend = []
                for t in range(NT):
                    if t == 1 and drain_q:
                        # prior sweep's drain emits here so its recip/mul ops
                        # queue BEHIND this sweep's first exps on DVE (the sc
                        # bank recycle path stays low-latency at sweep start)
                        drain_q.pop(0)()
                    # qkv generation woven just-in-time into sweep 0: K block
                    # m+1 lands 2 chunks before its scores need it, so a DMA
                    # wait parks the PE queue as late as possible
                    if si == 0:
                        m = t // 4
                        if t % 4 == 1 and m < 7:
                            emit_kt_block(m + 1)
                        elif t % 4 == 2 and m < 7:
                            emit_vt_block(m + 1)
                        elif t == 4:
                            emit_qt_pair(1)
                    elif si == 1:
                        if t == 0:
                            emit_qt_pair(2)
                        elif t == 4:
                            emit_qt_pair(3)
                        elif t == 8:
                            emit_qt_pair(4)
                        elif t == 12:
                            emit_qt_pair(6)
                    # sweeps >=2 have no qkv weaves, so the aux banks are only
                    # lightly used (12 proj chunks): borrow them for odd-t
                    # L-scores, deepening the scL rotation to ~4 chunks.  The
                    # score then waits exp(t-4) instead of exp(t-2), which
                    # absorbs the exp-queue jitter that was stalling the PE.
                    if si >= 2 and t % 2 == 1:
                        scL = psp.tile([128, 512], F32, tag="aux", name="scLx")
                    else:
                        scL = psp.tile([128, 512], F32, tag="scL", name="scL")
                    scR = psp.tile([128, 512], F32, tag="scR", name="scR")
                    for (u, ktt, slot, qcol, _ocol), sct in ((Lside, scL), (Rside, scR)):
                        nc.tensor.matmul(
                            sct[:, 0:512],
                            lhsT=ktt[slot : slot + 64, ts(t, 128)],
                            rhs=QT_sb[slot : slot + 64, ds(qcol, 512)],
                            start=True,
                            stop=True,
                        )
                    ptL = wkp.tile([128, 512], BF16, tag="ptL", name="ptL", bufs=18)
                    ptR = wkp.tile([128, 512], BF16, tag="ptR", name="ptR", bufs=18)
                    # no bias: the 2^0.5 factor vs the DVE branch's window is a
                    # per-unit global scale that cancels in the normalization
                    nc.scalar.activation(
                        ptL[:, 0:512],
                        scL[:, 0:512],
                        mybir.ActivationFunctionType.Exp,
                        scale=EXP_SCALE,
                    )
                    nc.vector._custom_dve(
                        expb,
                        out=ptR[:, 0:512].bitcast(mybir.dt.uint16),
                        in0=scR[:, 0:512],
                        in1=q0_sb[:],
                        s0=EXP_M,
                        s1=EXP_Q2,
                        imm2=EXP_Q1,
                    )
                    pend.append((t, ptL, ptR))
                    if t % 2 == 1:
                        # AVs lag the scores so their exp inputs are done at
                        # issue: every PE instruction is dep-free (throughput-
                        # bound, not exp-latency-bound).  Sweep 0 lags much
                        # deeper: the first V transposes cannot land before
                        # ~34us (gated behind the bulk input DMAs), so scores
                        # and qkv weaves fill the PE until V is ready instead
                        # of parking the queue on the first AV.
                        if len(pend) >= (16 if si == 0 else 6):
                            emit_av_pair(pend.pop(0), pend.pop(0))
                        pump(1)
                while len(pend) >= 2:
                    emit_av_pair(pend.pop(0), pend.pop(0))

                # drain: stage UNNORMALIZED O^T + the denominator row; the
                # host divides (softmax normalization is a per-query scalar
                # that commutes with the projection).  Deferred into the NEXT
                # sweep (t==1) so these copies don't block its exps.
                def emit_drain(sides=(Lside, Rside), avs=(avL, avR)):
                    for (u, _ktt, _slot, _qcol, ocol), av in zip(sides, avs):
                        _vb, _wpi, _yd, ybase = USPEC[u]
                        jc = (ocol + ybase) // 512 + (8 if u == "u2" else 0)
                        cp_alt[0] ^= 1
                        e1 = nc.scalar if cp_alt[0] else nc.vector
                        e2 = nc.vector if cp_alt[0] else nc.scalar
                        if e1 is nc.scalar:
                            nc.scalar.copy(
                                ot_tiles[u][0:64, ds(ocol, 512)], av[0:64, :]
                            )
                        else:
                            nc.vector.tensor_copy(
                                ot_tiles[u][0:64, ds(ocol, 512)], av[0:64, :]
                            )
                        if e2 is nc.scalar:
                            nc.scalar.copy(
                                den_sb[64:128, ds(jc * 512, 512)], av[64:128, :]
                            )
                        else:
                            nc.vector.tensor_copy(
                                den_sb[64:128, ds(jc * 512, 512)], av[64:128, :]
                            )
                        for oe in range(PO):
                            proj_q.append(
                                lambda tail=False, u=u, oe=oe, col=ocol: (
                                    emit_proj_chunk(u, oe, col, tail)
                                )
                            )

                drain_q.append(emit_drain)
            # last sweep's deferred drain
            while drain_q:
                drain_q.pop(0)()
            # ship the denominator row (one partition's worth)
            nc.sync.dma_start(den.ap(), den_sb[64:65, :])
            # tail: bridge the last sweep's drain latency with filler matmuls
            # that DEPEND on the drain's output (otherwise the scheduler
            # hoists them ahead of the drain and they fill nothing)
            tw_holder.append(psp.tile([128, 512], F32, tag="scR", name="tw"))
            for _ in range(10):
                nc.tensor.matmul(
                    tw_holder[0][:, 0:448],
                    lhsT=ot_u1[0:64, 0:128],
                    rhs=ot_u1[0:64, 1536:1984],
                    start=True, stop=True,
                )
            while proj_q:
                proj_q.pop(0)(tail=True)

    nc.compile()
    return nc


def _build():
    if "nc" not in _CACHE:
        nc = bacc.Bacc(None, target_bir_lowering=False, debug=False)
        _CACHE["nc"] = _emit(nc)
    return _CACHE["nc"]


def _prep_inputs(x, w_qkv, w_proj):
    bf = ml_dtypes.bfloat16
    xs = np.ascontiguousarray(x.reshape(S, D).T).astype(bf)  # [D, S]
    # chunk-major [pi, n, po*512+s]: element = xT[po*128+pi, n*512+s]
    x8 = np.ascontiguousarray(
        xs.reshape(PO, 128, NC8, 512).transpose(1, 2, 0, 3).reshape(128, NC8, PO * 512)
    )
    in_maps = []
    for c in range(NCORES):
        ha = c
        hb = 8 + c // 2
        bh = c % 2
        rows_q = lambda h: w_qkv[h * HD : (h + 1) * HD, :]
        rows_k = lambda h: w_qkv[D + h * HD : D + (h + 1) * HD, :]
        rows_v = lambda h: w_qkv[2 * D + h * HD : 2 * D + (h + 1) * HD, :]
        qs = SCALE * LOG2E_128
        wq_c = np.concatenate([rows_q(ha) * qs, rows_q(hb) * qs], 0).T  # [D, 128]
        wk_c = np.concatenate([rows_k(ha), rows_k(hb)], 0).T
        wv_c = np.concatenate([rows_v(ha), rows_v(hb)], 0).T
        wp_c = np.stack(
            [w_proj[:, ha * HD : (ha + 1) * HD].T, w_proj[:, hb * HD : (hb + 1) * HD].T],
            axis=1,
        )  # [64, 2, D]
        shuf = lambda w: np.ascontiguousarray(
            w.reshape(PO, 128, 128).transpose(1, 0, 2).reshape(128, PO * 128)
        ).astype(bf)
        # local chunk j = global chunk (j + 4*bh) % 8, so head-B's query rows
        # are always local chunks 0-3 (key order is irrelevant to softmax)
        x8c = np.roll(x8, -4 * bh, axis=1) if bh else x8
        in_maps.append(
            {
                "xT": np.ascontiguousarray(x8c),
                "wq": shuf(wq_c),
                "wk": shuf(wk_c),
                "wv": shuf(wv_c),
                "wp": np.ascontiguousarray(wp_c.reshape(64, 2 * D)).astype(bf),
            }
        )
    return in_maps


def _combine(results, b_proj):
    yT = np.zeros((D, S), np.float32)
    for c in range(NCORES):
        bh = c % 2
        dn = results[c]["den"].astype(np.float32).reshape(12, 512)
        ya = results[c]["yTa"].astype(np.float32)  # [128, 8, 6*512] local order
        ya = ya.reshape(128, NC8, PO, 512) / dn[None, 0:8, None, :]
        if bh:
            ya = np.roll(ya, 4 * bh, axis=1)  # local block j -> global (j+4)%8
        yT += ya.transpose(2, 0, 1, 3).reshape(D, S)
        yb = results[c]["yTb"].astype(np.float32)  # [128, 4, 6*512]
        yb = yb.reshape(128, 4, PO, 512) / dn[None, 8:12, None, :]
        yT[:, bh * SU : (bh + 1) * SU] += yb.transpose(2, 0, 1, 3).reshape(D, SU)
    y = yT.T + b_proj.astype(np.float32)[None, :]
    return y.reshape(1, 64, 64, D).astype(np.float32)


def kernel(x, w_qkv, w_proj, b_proj, _trace=False, _trace_kwargs=None):
    x = np.asarray(x, np.float32)
    w_qkv = np.asarray(w_qkv, np.float32)
    w_proj = np.asarray(w_proj, np.float32)
    b_proj = np.asarray(b_proj, np.float32)

    nc = _build()
    in_maps = _prep_inputs(x, w_qkv, w_proj)
    res = run_bass_kernel_spmd(
        nc, in_maps, core_ids=list(range(NCORES)), trace=_trace,
        **(_trace_kwargs or {}),
    )
    out = _combine(res.results, b_proj)
    if _trace:
        return out, res
    return out
